# revision 2
# baseline (speedup 1.0000x reference)
"""DeepSeek layer (MLA attention + shared/routed MoE) on 8 TRN2 NeuronCores.

Data-parallel over tokens: core c handles batch c//4, tokens [(c%4)*256, ...).
Activations live feature-major [feature, token] on device; host pre-transposes
weights (bf16) and precomputes the first rmsnorm (depends only on input X).
Router logits are computed in fp32 so top-2 expert selection matches the
reference; expert matmuls run in bf16.
"""

import numpy as np
import ml_dtypes

import concourse.bass as bass
import concourse.tile as tile
from concourse import bacc, mybir
from concourse.bass_utils import run_bass_kernel_spmd
from concourse.masks import make_identity

BF16 = mybir.dt.bfloat16
F32 = mybir.dt.float32
F8 = mybir.dt.float8e4
AX = mybir.AxisListType.X
ALU = mybir.AluOpType
ACTF = mybir.ActivationFunctionType
DR = mybir.MatmulPerfMode.DoubleRow

# fp8 scale factors: weights x64, activations x16, hidden x8
WS = 64.0
XS = 16.0
HS = 8.0

P = 128
D = 1024
KD = D // P          # 8 feature chunks
S = 1024             # keys per batch
TQ = 256             # query tokens per core
H = 4
DC = 256             # compressed kv dim == dk
F = 1024
KF = F // P
E = 8
EPS = 1e-6
SCALE = 1.0 / 16.0   # 1/sqrt(dk)

_CACHE = {}


def _r(ap, n=None):
    """Host-permuted DRAM [P, C*N] -> [P, C, N] view (contiguous)."""
    c = ap.shape[-1]
    n = n if n is not None else c // KD
    return ap.rearrange("p (k n) -> p k n", n=n)


def build_program():
    nc = bacc.Bacc(None)

    # All tensors are host-permuted to partition-major [P, chunks*N] so each
    # DMA is one contiguous segment per partition (descriptor-rate matters).
    din = {}
    for name, shape, dt in [
        ("nxt", [P, KD * S], BF16),
        ("nxq", [P, KD * TQ], BF16),
        ("xt", [P, KD * TQ], F32),
        ("wq", [P, KD * D], BF16),
        ("wkc", [P, KD * DC], BF16),
        ("wvc", [P, KD * DC], BF16),
        ("wo", [P, KD * D], BF16),
        ("wr", [P, KD * E], F32),
        ("ebias", [1, E], F32),
        ("sw1", [P, KD * F], BF16),
        ("sw3", [P, KD * F], BF16),
        ("sw2", [P, KF * D], BF16),
        ("ew1", [E, P, KD * F], BF16),
        ("ew3", [E, P, KD * F], BF16),
        ("ew2", [E, P, KF * D], BF16),
    ]:
        din[name] = nc.dram_tensor(name, shape, dt, kind="ExternalInput")
    outt = nc.dram_tensor("outt", [P, KD * TQ], F32, kind="ExternalOutput")

    with tile.TileContext(nc) as tc:
        with (
            tc.tile_pool(name="const", bufs=1) as const,
            tc.tile_pool(name="persist", bufs=1) as persist,
        ):
            ones_bf = const.tile([P, 1], BF16)
            nc.vector.memset(ones_bf, 1.0)
            ones_cf = const.tile([P, 1], F32)
            nc.vector.memset(ones_cf, 1.0)
            ones_row = const.tile([1, P], F32)
            nc.vector.memset(ones_row, 1.0)
            eps1 = const.tile([1, 1], F32)
            nc.vector.memset(eps1, EPS)
            ident = const.tile([P, P], F32)
            make_identity(nc, ident)
            ebias_b = const.tile([P, E], F32)
            nc.sync.dma_start(ebias_b, din["ebias"][:].to_broadcast([P, E]))

            xpT = persist.tile([P, KD, TQ], F32)      # X' = X + attn out
            nx2 = persist.tile([P, KD, TQ], BF16)     # rmsnorm2(X') bf16
            cb = persist.tile([P, E, TQ], F32)        # combine weights bcast
            spec = persist.tile([P, KD, TQ], F32)     # shared+experts accum
            outT = persist.tile([P, KD, TQ], F32)

            # shared-expert weights, prefetched during attention so the MoE
            # phase starts without a DMA stall (DMAs issued after the
            # attention inputs below)
            w1_0 = persist.tile([P, KD, F], BF16)
            w3_0 = persist.tile([P, KD, F], BF16)
            w2_0 = persist.tile([P, KF, D], BF16)

            # ---------------- attention ----------------
            with (
                tc.tile_pool(name="ainp", bufs=1) as ainp,
                tc.tile_pool(name="awts", bufs=1) as awts,
                tc.tile_pool(name="wostr", bufs=2) as wostr,
                tc.tile_pool(name="aact", bufs=1) as aact,
                tc.tile_pool(name="psA", bufs=3, space="PSUM") as psA,
                tc.tile_pool(name="psD", bufs=2, space="PSUM") as psD,
            ):
                # DMA issue order: Q-path operands first so PE starts early
                nxq = ainp.tile([P, KD, TQ], BF16)
                nc.sync.dma_start(nxq, _r(din["nxq"][:], TQ))
                wkc = awts.tile([P, KD, DC], BF16)
                nc.sync.dma_start(wkc, _r(din["wkc"][:], DC))
                nxt = ainp.tile([P, KD, S], BF16)
                nc.sync.dma_start(nxt, _r(din["nxt"][:], S))
                wvc = awts.tile([P, KD, DC], BF16)
                nc.sync.dma_start(wvc, _r(din["wvc"][:], DC))
                xt = ainp.tile([P, KD, TQ], F32)
                nc.sync.dma_start(xt, _r(din["xt"][:], TQ))
                wr = awts.tile([P, KD, E], F32)
                nc.sync.dma_start(wr, _r(din["wr"][:], E))

                qT = aact.tile([P, KD, TQ], BF16)
                kcT = aact.tile([P, 2, S], BF16)
                vc = aact.tile([P, KD, DC], BF16)
                probs = aact.tile([P, KD, H, TQ], BF16)
                rcpd = aact.tile([1, H, TQ], F32)
                rcpb = aact.tile([P, H, TQ], F32)
                oT = aact.tile([P, KD, TQ], BF16)

                # Q^T [d, tq] = Wq @ nxq  (wq loaded in two halves so the
                # first matmuls can start after ~1MB of DMA)
                wq = awts.tile([P, KD, D], BF16)
                nc.sync.dma_start(wq[:, :, :512], _r(din["wq"][:], D)[:, :, :512])
                nc.sync.dma_start(wq[:, :, 512:], _r(din["wq"][:], D)[:, :, 512:])
                for m in range(KD):
                    ps = psA.tile([P, 512], F32, tag="mm")
                    for k in range(KD):
                        nc.tensor.matmul(
                            ps[:, :TQ], wq[:, k, m * P:(m + 1) * P],
                            nxq[:, k, :], start=(k == 0), stop=(k == KD - 1))
                    nc.scalar.copy(qT[:, m, :], ps[:, :TQ])

                # Kc^T [dc, S] = Wkc @ nxt
                for m in range(2):
                    for n2 in range(2):
                        ps = psA.tile([P, 512], F32, tag="mm")
                        for k in range(KD):
                            nc.tensor.matmul(
                                ps, wkc[:, k, m * P:(m + 1) * P],
                                nxt[:, k, n2 * 512:(n2 + 1) * 512],
                                start=(k == 0), stop=(k == KD - 1))
                        nc.scalar.copy(kcT[:, m, n2 * 512:(n2 + 1) * 512], ps)

                # Vc [S, dc] = nxt^T @ Wvc^T  (keys-major)
                for kc in range(KD):
                    ps = psA.tile([P, 512], F32, tag="mm")
                    for k in range(KD):
                        nc.tensor.matmul(
                            ps[:, :DC], nxt[:, k, kc * P:(kc + 1) * P],
                            wvc[:, k, :], start=(k == 0), stop=(k == KD - 1))
                    nc.scalar.copy(vc[:, kc, :], ps[:, :DC])

                # scores^T + exp (keys-major, no max-sub needed)
                for h in range(H):
                    for kc in range(KD):
                        ps = psA.tile([P, 512], F32, tag="mm")
                        for m in range(2):
                            nc.tensor.matmul(
                                ps[:, :TQ], kcT[:, m, kc * P:(kc + 1) * P],
                                qT[:, 2 * h + m, :], start=(m == 0), stop=(m == 1))
                        nc.scalar.activation(
                            probs[:, kc, h, :], ps[:, :TQ], ACTF.Exp, scale=SCALE)

                # prefetch shared-expert weights now: attention inputs are
                # in flight, MoE phase is ~50us away
                nc.sync.dma_start(w1_0, _r(din["sw1"][:], F))
                nc.sync.dma_start(w3_0, _r(din["sw3"][:], F))
                nc.sync.dma_start(w2_0, _r(din["sw2"][:], D))

                # softmax denominators + reciprocal + broadcast
                for h in range(H):
                    psd = psD.tile([1, TQ], F32, tag="aux")
                    for kc in range(KD):
                        nc.tensor.matmul(
                            psd, ones_bf[:, 0:1], probs[:, kc, h, :],
                            start=(kc == 0), stop=(kc == KD - 1))
                    nc.vector.reciprocal(rcpd[0:1, h, :], psd)
                    psb = psD.tile([P, TQ], F32, tag="aux")
                    nc.tensor.matmul(psb, ones_row, rcpd[0:1, h, :],
                                     start=True, stop=True)
                    nc.scalar.copy(rcpb[:, h, :], psb)

                # out_h^T = Vc^T @ probs^T, normalized per token
                for h in range(H):
                    for m in range(2):
                        ps = psA.tile([P, 512], F32, tag="mm")
                        for kc in range(KD):
                            nc.tensor.matmul(
                                ps[:, :TQ], vc[:, kc, m * P:(m + 1) * P],
                                probs[:, kc, h, :],
                                start=(kc == 0), stop=(kc == KD - 1))
                        nc.vector.tensor_mul(
                            out=oT[:, 2 * h + m, :], in0=ps[:, :TQ],
                            in1=rcpb[:, h, :])

                # attn proj + residual: X' = Wo @ O + X  (wo streamed)
                for m in range(KD):
                    wo_m = wostr.tile([P, KD, P], BF16, tag="wo")
                    nc.sync.dma_start(wo_m, _r(din["wo"][:], D)[:, :, m * P:(m + 1) * P])
                    ps = psA.tile([P, 512], F32, tag="mm")
                    for k in range(KD):
                        nc.tensor.matmul(
                            ps[:, :TQ], wo_m[:, k, :],
                            oT[:, k, :], start=(k == 0), stop=(k == KD - 1))
                    nc.vector.tensor_add(
                        out=xpT[:, m, :], in0=ps[:, :TQ], in1=xt[:, m, :])

                # ---------------- rmsnorm2 + router ----------------
                sq = aact.tile([P, KD, TQ], F32)
                rs = aact.tile([1, TQ], F32)
                sd = aact.tile([1, TQ], F32)
                rsb = aact.tile([P, TQ], F32)
                rstok = aact.tile([P, 2], F32)
                lg = aact.tile([P, 2, E], F32)
                comb = aact.tile([P, 2, E], F32)
                cT = aact.tile([E, TQ], F32)

                for m in range(KD):
                    nc.scalar.square(sq[:, m, :], xpT[:, m, :])
                psq = psD.tile([1, TQ], F32, tag="aux")
                for k in range(KD):
                    nc.tensor.matmul(psq, ones_cf[:, 0:1], sq[:, k, :],
                                     start=(k == 0), stop=(k == KD - 1))
                nc.scalar.activation(sd[0:1, :], psq, ACTF.Sqrt,
                                     bias=eps1[0:1, :], scale=1.0 / D)
                nc.vector.reciprocal(rs[0:1, :], sd[0:1, :])
                psb = psD.tile([P, TQ], F32, tag="aux")
                nc.tensor.matmul(psb, ones_row, rs[0:1, :], start=True, stop=True)
                nc.scalar.copy(rsb, psb)
                for m in range(KD):
                    nc.vector.tensor_mul(out=nx2[:, m, :], in0=xpT[:, m, :],
                                         in1=rsb)

                # rs in token-major via PE transpose (rows of rsb^T are const)
                for t in range(2):
                    pst = psA.tile([P, 512], F32, tag="mm")
                    nc.tensor.transpose(pst[:, :P], rsb[:, t * P:(t + 1) * P],
                                        ident)
                    nc.scalar.copy(rstok[:, t:t + 1], pst[:, 0:1])

                # logits (token-major, fp32): (X'^T_chunk^T @ Wr^T) * rs + bias
                for t in range(2):
                    psr = psD.tile([P, E], F32, tag="aux")
                    for k in range(KD):
                        nc.tensor.matmul(
                            psr, xpT[:, k, t * P:(t + 1) * P], wr[:, k, :],
                            start=(k == 0), stop=(k == KD - 1))
                    nc.vector.scalar_tensor_tensor(
                        out=lg[:, t, :], in0=psr, scalar=rstok[:, t:t + 1],
                        in1=ebias_b, op0=ALU.mult, op1=ALU.add)

                # softmax over experts + top-2 renormalized combine weights
                for t in range(2):
                    lgt = lg[:, t, :]
                    m_s = aact.tile([P, 1], F32, tag="sm", name=f"m_{t}")
                    nc.vector.reduce_max(m_s, lgt, axis=AX)
                    negm = aact.tile([P, 1], F32, tag="sm2", name=f"nm_{t}")
                    nc.vector.tensor_scalar_mul(negm, m_s, -1.0)
                    e_s = aact.tile([P, E], F32, tag="sm3", name=f"e_{t}")
                    den = aact.tile([P, 1], F32, tag="sm4", name=f"d_{t}")
                    nc.scalar.activation(e_s, lgt, ACTF.Exp, bias=negm,
                                         accum_out=den)
                    rcp = aact.tile([P, 1], F32, tag="sm5", name=f"r_{t}")
                    nc.vector.reciprocal(rcp, den)
                    rw = aact.tile([P, E], F32, tag="sm6", name=f"rw_{t}")
                    nc.vector.tensor_scalar_mul(rw, e_s, rcp)
                    m1 = aact.tile([P, 1], F32, tag="sm7", name=f"m1_{t}")
                    nc.vector.reduce_max(m1, rw, axis=AX)
                    mask1 = aact.tile([P, E], F32, tag="sm8", name=f"k1_{t}")
                    nc.vector.tensor_scalar(mask1, rw, m1, None, ALU.is_ge)
                    rw2 = aact.tile([P, E], F32, tag="sm9", name=f"rw2_{t}")
                    nc.vector.scalar_tensor_tensor(
                        out=rw2, in0=mask1, scalar=-10.0, in1=rw,
                        op0=ALU.mult, op1=ALU.add)
                    m2 = aact.tile([P, 1], F32, tag="sm10", name=f"m2_{t}")
                    nc.vector.reduce_max(m2, rw2, axis=AX)
                    masktop = aact.tile([P, E], F32, tag="sm11", name=f"kt_{t}")
                    nc.vector.tensor_scalar(masktop, rw, m2, None, ALU.is_ge)
                    er = aact.tile([P, E], F32, tag="sm12", name=f"er_{t}")
                    nc.scalar.activation(er, rw, ACTF.Exp)
                    erm = aact.tile([P, E], F32, tag="sm13", name=f"em_{t}")
                    nc.vector.tensor_mul(out=erm, in0=er, in1=masktop)
                    den2 = aact.tile([P, 1], F32, tag="sm14", name=f"d2_{t}")
                    nc.vector.reduce_sum(den2, erm, axis=AX)
                    rcp2 = aact.tile([P, 1], F32, tag="sm15", name=f"r2_{t}")
                    nc.vector.reciprocal(rcp2, den2)
                    nc.vector.tensor_scalar_mul(comb[:, t, :], erm, rcp2)

                # combine^T [E, TQ] via PE transpose, then row-broadcast
                for t in range(2):
                    pst = psA.tile([P, 512], F32, tag="mm")
                    nc.tensor.transpose(pst[:E, :P], comb[:, t, :], ident)
                    nc.scalar.copy(cT[:, t * P:(t + 1) * P], pst[:E, :P])
                with tc.tile_pool(name="dbounce", bufs=1, space="DRAM") as dbp:
                    cdram = dbp.tile([E, TQ], F32)
                    nc.sync.dma_start(cdram, cT)
                    for e in range(E):
                        nc.sync.dma_start(
                            cb[:, e, :],
                            cdram[e:e + 1, :].to_broadcast([P, TQ]))

            # ---------------- shared expert + 8 routed experts ----------------
            with (
                tc.tile_pool(name="wmoe", bufs=2) as wmoe,
                tc.tile_pool(name="mact", bufs=2) as mact,
                tc.tile_pool(name="psM", bufs=5, space="PSUM") as psM,
                tc.tile_pool(name="psO", bufs=3, space="PSUM") as psO,
            ):
                for u in range(E + 1):
                    if u == 0:
                        w1, w3, w2 = w1_0, w3_0, w2_0
                    else:
                        w1 = wmoe.tile([P, KD, F], BF16, tag="w1")
                        nc.sync.dma_start(w1, _r(din["ew1"][u - 1], F))
                        w3 = wmoe.tile([P, KD, F], BF16, tag="w3")
                        nc.sync.dma_start(w3, _r(din["ew3"][u - 1], F))
                        w2 = wmoe.tile([P, KF, D], BF16, tag="w2")
                        nc.sync.dma_start(w2, _r(din["ew2"][u - 1], D))

                    hp = mact.tile([P, KF, TQ], BF16, tag="hp")
                    for m in range(KF):
                        psa = psM.tile([P, TQ], F32, tag="ab")
                        for k in range(KD):
                            nc.tensor.matmul(
                                psa, w1[:, k, m * P:(m + 1) * P], nx2[:, k, :],
                                start=(k == 0), stop=(k == KD - 1))
                        sa = mact.tile([P, TQ], BF16, tag="sa")
                        nc.scalar.activation(sa, psa, ACTF.Silu)
                        psb2 = psM.tile([P, TQ], F32, tag="ab")
                        for k in range(KD):
                            nc.tensor.matmul(
                                psb2, w3[:, k, m * P:(m + 1) * P], nx2[:, k, :],
                                start=(k == 0), stop=(k == KD - 1))
                        if u == 0:
                            nc.vector.tensor_mul(out=hp[:, m, :], in0=psb2,
                                                 in1=sa)
                        else:
                            tmp = mact.tile([P, TQ], BF16, tag="tmp")
                            nc.vector.tensor_mul(out=tmp, in0=sa,
                                                 in1=cb[:, u - 1, :])
                            nc.vector.tensor_mul(out=hp[:, m, :], in0=psb2,
                                                 in1=tmp)
                    for m in range(KD):
                        pso = psO.tile([P, TQ], F32, tag="eo")
                        for k in range(KF):
                            nc.tensor.matmul(
                                pso, w2[:, k, m * P:(m + 1) * P], hp[:, k, :],
                                start=(k == 0), stop=(k == KF - 1))
                        if u == 0:
                            nc.vector.tensor_copy(spec[:, m, :], pso)
                        elif u < E:
                            nc.vector.tensor_add(out=spec[:, m, :],
                                                 in0=spec[:, m, :], in1=pso)
                        else:
                            nc.vector.tensor_add(out=spec[:, m, :],
                                                 in0=spec[:, m, :], in1=pso)
                            nc.vector.tensor_add(out=outT[:, m, :],
                                                 in0=spec[:, m, :],
                                                 in1=xpT[:, m, :])
                            nc.sync.dma_start(
                                _r(outt[:], TQ)[:, m, :], outT[:, m, :])

    nc.finalize()
    return nc


def _prep_inputs(inputs):
    bf = ml_dtypes.bfloat16
    X = np.asarray(inputs["X"], np.float32)
    g1 = np.asarray(inputs["g1"], np.float32)
    rs1 = 1.0 / np.sqrt(np.mean(X * X, axis=-1, keepdims=True) + EPS)
    nx = X * rs1 * g1                      # [2, S, D] fp32
    nxT = np.ascontiguousarray(np.transpose(nx, (0, 2, 1))).astype(bf)
    XT = np.ascontiguousarray(np.transpose(X, (0, 2, 1)))

    def pm(a):
        """[C*P, N] -> partition-major [P, C*N]."""
        cp, n = a.shape
        return np.ascontiguousarray(
            a.reshape(cp // P, P, n).swapaxes(0, 1).reshape(P, -1))

    def t2(a):
        return pm(np.ascontiguousarray(np.asarray(a, np.float32).T))

    def t3(a):
        a = np.transpose(np.asarray(a, np.float32), (0, 2, 1))
        return np.stack([pm(np.ascontiguousarray(a[e])) for e in range(E)])

    shared = {
        "wq": t2(inputs["Wq"]).astype(bf),
        "wkc": t2(inputs["Wkc"]).astype(bf),
        "wvc": t2(inputs["Wvc"]).astype(bf),
        "wo": t2(inputs["Wo"]).astype(bf),
        "wr": t2(inputs["Wr"]).astype(np.float32),
        "ebias": np.asarray(inputs["expert_bias"],
                            np.float32).reshape(1, E),
        "sw1": t2(inputs["sW1"]).astype(bf),
        "sw3": t2(inputs["sW3"]).astype(bf),
        "sw2": t2(inputs["sW2"]).astype(bf),
        "ew1": t3(inputs["eW1"]).astype(bf),
        "ew3": t3(inputs["eW3"]).astype(bf),
        "ew2": t3(inputs["eW2"]).astype(bf),
    }
    in_maps = []
    for c in range(8):
        b, q0 = c // 4, (c % 4) * TQ
        m = dict(shared)

        def pm(a):
            cp, n = a.shape
            return np.ascontiguousarray(
                a.reshape(cp // P, P, n).swapaxes(0, 1).reshape(P, -1))

        m["nxt"] = pm(nxT[b])
        m["nxq"] = pm(np.ascontiguousarray(nxT[b][:, q0:q0 + TQ]))
        m["xt"] = pm(np.ascontiguousarray(XT[b][:, q0:q0 + TQ]))
        in_maps.append(m)
    return in_maps


def run_on_device(inputs, trace=False):
    if "nc" not in _CACHE:
        _CACHE["nc"] = build_program()
    nc = _CACHE["nc"]
    in_maps = _prep_inputs(inputs)
    res = run_bass_kernel_spmd(nc, in_maps, core_ids=list(range(8)),
                               trace=trace)
    out = np.empty((2, S, D), np.float32)
    for c in range(8):
        b, q0 = c // 4, (c % 4) * TQ
        ot = res.results[c]["outt"].reshape(P, KD, TQ).swapaxes(0, 1)
        out[b, q0:q0 + TQ, :] = ot.reshape(D, TQ).T
    return out, res


def kernel(**inputs):
    out, _ = run_on_device(inputs, trace=False)
    return out



# revision 8
# speedup vs baseline: 1.3427x; 1.3427x over previous
"""DeepSeek layer (MLA attention + shared/routed MoE) on 8 TRN2 NeuronCores.

Data-parallel over tokens: core c handles batch c//4, tokens [(c%4)*256, ...).
Activations live feature-major [feature, token] on device; host pre-transposes
weights (bf16) and precomputes the first rmsnorm (depends only on input X).
Router logits are computed in fp32 so top-2 expert selection matches the
reference; expert matmuls run in bf16.
"""

import numpy as np
import ml_dtypes

import concourse.bass as bass
import concourse.tile as tile
from concourse import bacc, mybir
from concourse.bass_utils import run_bass_kernel_spmd
from concourse.masks import make_identity

BF16 = mybir.dt.bfloat16
F32 = mybir.dt.float32
F8 = mybir.dt.float8e4
AX = mybir.AxisListType.X
ALU = mybir.AluOpType
ACTF = mybir.ActivationFunctionType
DR = mybir.MatmulPerfMode.DoubleRow

# fp8 scale factors: weights x64, activations x16, hidden x8
WS = 64.0
XS = 16.0
HS = 8.0

P = 128
D = 1024
KD = D // P          # 8 feature chunks
S = 1024             # keys per batch
TQ = 256             # query tokens per core
H = 4
DC = 256             # compressed kv dim == dk
F = 1024
KF = F // P
E = 8
EPS = 1e-6
SCALE = 1.0 / 16.0   # 1/sqrt(dk)

_CACHE = {}


def _r(ap, n=None):
    """Host-permuted DRAM [P, C*N] -> [P, C, N] view (contiguous)."""
    c = ap.shape[-1]
    n = n if n is not None else c // KD
    return ap.rearrange("p (k n) -> p k n", n=n)


def build_program():
    nc = bacc.Bacc(None)

    # All tensors are host-permuted to partition-major [P, chunks*N] so each
    # DMA is one contiguous segment per partition (descriptor-rate matters).
    din = {}
    for name, shape, dt in [
        ("nxt", [P, KD * S], BF16),
        ("nxq", [P, KD * TQ], BF16),
        ("xt", [P, KD * TQ], F32),
        ("wq", [P, KD * D], BF16),
        ("wkc", [P, KD * DC], BF16),
        ("wvc", [P, KD * DC], BF16),
        ("wo", [P, KD * D], BF16),
        ("wr", [P, KD * E], F32),
        ("ebias", [1, E], F32),
        ("sw1", [P, KD * F], F8),
        ("sw3", [P, KD * F], F8),
        ("sw2", [P, KF * D], F8),
        ("ew1", [E, P, KD * F], F8),
        ("ew3", [E, P, KD * F], F8),
        ("ew2", [E, P, KF * D], F8),
    ]:
        din[name] = nc.dram_tensor(name, shape, dt, kind="ExternalInput")
    outt = nc.dram_tensor("outt", [P, KD * TQ], F32, kind="ExternalOutput")

    with tile.TileContext(nc) as tc:
        with (
            tc.tile_pool(name="const", bufs=1) as const,
            tc.tile_pool(name="persist", bufs=1) as persist,
        ):
            ones_bf = const.tile([P, 1], BF16)
            nc.vector.memset(ones_bf, 1.0)
            ones_cf = const.tile([P, 1], F32)
            nc.vector.memset(ones_cf, 1.0)
            ones_row = const.tile([1, P], F32)
            nc.vector.memset(ones_row, 1.0)
            eps1 = const.tile([1, 1], F32)
            nc.vector.memset(eps1, EPS)
            ident = const.tile([P, P], F32)
            make_identity(nc, ident)
            ebias_b = const.tile([P, E], F32)
            nc.sync.dma_start(ebias_b, din["ebias"][:].to_broadcast([P, E]))

            xpT = persist.tile([P, KD, TQ], F32)      # X' = X + attn out
            nx2 = persist.tile([P, KD, TQ], F8)       # rmsnorm2(X')*XS fp8
            cb = persist.tile([P, E, TQ], F32)        # combine weights bcast
            spec = persist.tile([P, KD, TQ], F32)     # shared+experts accum
            outT = persist.tile([P, KD, TQ], F32)

            # shared-expert weights, prefetched during attention so the MoE
            # phase starts without a DMA stall (DMAs issued after the
            # attention inputs below)
            w1_0 = persist.tile([P, KD, F], F8)
            w3_0 = persist.tile([P, KD, F], F8)
            w2_0 = persist.tile([P, KF, D], F8)

            # ---------------- attention ----------------
            with (
                tc.tile_pool(name="ainp", bufs=1) as ainp,
                tc.tile_pool(name="awts", bufs=1) as awts,
                tc.tile_pool(name="wostr", bufs=2) as wostr,
                tc.tile_pool(name="aact", bufs=1) as aact,
                tc.tile_pool(name="psA", bufs=3, space="PSUM") as psA,
                tc.tile_pool(name="psD", bufs=2, space="PSUM") as psD,
            ):
                # DMA issue order: Q-path operands first so PE starts early
                nxq = ainp.tile([P, KD, TQ], BF16)
                nc.sync.dma_start(nxq, _r(din["nxq"][:], TQ))
                wkc = awts.tile([P, KD, DC], BF16)
                nc.sync.dma_start(wkc, _r(din["wkc"][:], DC))
                nxt = ainp.tile([P, KD, S], BF16)
                nc.sync.dma_start(nxt, _r(din["nxt"][:], S))
                wvc = awts.tile([P, KD, DC], BF16)
                nc.sync.dma_start(wvc, _r(din["wvc"][:], DC))
                xt = ainp.tile([P, KD, TQ], F32)
                nc.sync.dma_start(xt, _r(din["xt"][:], TQ))
                wr = awts.tile([P, KD, E], F32)
                nc.sync.dma_start(wr, _r(din["wr"][:], E))

                qT = aact.tile([P, KD, TQ], BF16)
                kcT = aact.tile([P, 2, S], BF16)
                vc = aact.tile([P, KD, DC], BF16)
                probs = aact.tile([P, KD, H, TQ], BF16)
                rcpd = aact.tile([1, H, TQ], F32)
                rcpb = aact.tile([P, H, TQ], F32)
                oT = aact.tile([P, KD, TQ], BF16)

                # Q^T [d, tq] = Wq @ nxq  (wq loaded in two halves so the
                # first matmuls can start after ~1MB of DMA)
                wq = awts.tile([P, KD, D], BF16)
                nc.sync.dma_start(wq[:, :, :512], _r(din["wq"][:], D)[:, :, :512])
                nc.sync.dma_start(wq[:, :, 512:], _r(din["wq"][:], D)[:, :, 512:])
                for m in range(KD):
                    ps = psA.tile([P, 512], F32, tag="mm")
                    for k in range(KD):
                        nc.tensor.matmul(
                            ps[:, :TQ], wq[:, k, m * P:(m + 1) * P],
                            nxq[:, k, :], start=(k == 0), stop=(k == KD - 1))
                    nc.scalar.copy(qT[:, m, :], ps[:, :TQ])

                # Kc^T [dc, S] = Wkc @ nxt
                for m in range(2):
                    for n2 in range(2):
                        ps = psA.tile([P, 512], F32, tag="mm")
                        for k in range(KD):
                            nc.tensor.matmul(
                                ps, wkc[:, k, m * P:(m + 1) * P],
                                nxt[:, k, n2 * 512:(n2 + 1) * 512],
                                start=(k == 0), stop=(k == KD - 1))
                        nc.scalar.copy(kcT[:, m, n2 * 512:(n2 + 1) * 512], ps)

                # Vc [S, dc] = nxt^T @ Wvc^T  (keys-major)
                for kc in range(KD):
                    ps = psA.tile([P, 512], F32, tag="mm")
                    for k in range(KD):
                        nc.tensor.matmul(
                            ps[:, :DC], nxt[:, k, kc * P:(kc + 1) * P],
                            wvc[:, k, :], start=(k == 0), stop=(k == KD - 1))
                    nc.scalar.copy(vc[:, kc, :], ps[:, :DC])

                # scores^T + exp (keys-major, no max-sub needed)
                for h in range(H):
                    for kc in range(KD):
                        ps = psA.tile([P, 512], F32, tag="mm")
                        for m in range(2):
                            nc.tensor.matmul(
                                ps[:, :TQ], kcT[:, m, kc * P:(kc + 1) * P],
                                qT[:, 2 * h + m, :], start=(m == 0), stop=(m == 1))
                        nc.scalar.activation(
                            probs[:, kc, h, :], ps[:, :TQ], ACTF.Exp, scale=SCALE)

                # prefetch shared-expert weights now: attention inputs are
                # in flight, MoE phase is ~50us away
                nc.sync.dma_start(w1_0, _r(din["sw1"][:], F))
                nc.sync.dma_start(w3_0, _r(din["sw3"][:], F))
                nc.sync.dma_start(w2_0, _r(din["sw2"][:], D))

                # softmax denominators + reciprocal + broadcast
                for h in range(H):
                    psd = psD.tile([1, TQ], F32, tag="aux")
                    for kc in range(KD):
                        nc.tensor.matmul(
                            psd, ones_bf[:, 0:1], probs[:, kc, h, :],
                            start=(kc == 0), stop=(kc == KD - 1))
                    nc.vector.reciprocal(rcpd[0:1, h, :], psd)
                    psb = psD.tile([P, TQ], F32, tag="aux")
                    nc.tensor.matmul(psb, ones_row, rcpd[0:1, h, :],
                                     start=True, stop=True)
                    nc.scalar.copy(rcpb[:, h, :], psb)

                # out_h^T = Vc^T @ probs^T, normalized per token
                for h in range(H):
                    for m in range(2):
                        ps = psA.tile([P, 512], F32, tag="mm")
                        for kc in range(KD):
                            nc.tensor.matmul(
                                ps[:, :TQ], vc[:, kc, m * P:(m + 1) * P],
                                probs[:, kc, h, :],
                                start=(kc == 0), stop=(kc == KD - 1))
                        nc.vector.tensor_mul(
                            out=oT[:, 2 * h + m, :], in0=ps[:, :TQ],
                            in1=rcpb[:, h, :])

                # attn proj + residual: X' = Wo @ O + X  (wo streamed)
                for m in range(KD):
                    wo_m = wostr.tile([P, KD, P], BF16, tag="wo")
                    nc.sync.dma_start(wo_m, _r(din["wo"][:], D)[:, :, m * P:(m + 1) * P])
                    ps = psA.tile([P, 512], F32, tag="mm")
                    for k in range(KD):
                        nc.tensor.matmul(
                            ps[:, :TQ], wo_m[:, k, :],
                            oT[:, k, :], start=(k == 0), stop=(k == KD - 1))
                    nc.vector.tensor_add(
                        out=xpT[:, m, :], in0=ps[:, :TQ], in1=xt[:, m, :])

                # ---------------- rmsnorm2 + router ----------------
                sq = aact.tile([P, KD, TQ], F32)
                rs = aact.tile([1, TQ], F32)
                sd = aact.tile([1, TQ], F32)
                rsb = aact.tile([P, TQ], F32)
                rstok = aact.tile([P, 2], F32)
                lg = aact.tile([P, 2, E], F32)
                comb = aact.tile([P, 2, E], F32)
                cT = aact.tile([E, TQ], F32)

                for m in range(KD):
                    nc.scalar.square(sq[:, m, :], xpT[:, m, :])
                psq = psD.tile([1, TQ], F32, tag="aux")
                for k in range(KD):
                    nc.tensor.matmul(psq, ones_cf[:, 0:1], sq[:, k, :],
                                     start=(k == 0), stop=(k == KD - 1))
                nc.scalar.activation(sd[0:1, :], psq, ACTF.Sqrt,
                                     bias=eps1[0:1, :], scale=1.0 / D)
                nc.vector.reciprocal(rs[0:1, :], sd[0:1, :])
                psb = psD.tile([P, TQ], F32, tag="aux")
                nc.tensor.matmul(psb, ones_row, rs[0:1, :], start=True, stop=True)
                nc.scalar.copy(rsb, psb)
                for m in range(KD):
                    nc.vector.scalar_tensor_tensor(
                        out=nx2[:, m, :], in0=xpT[:, m, :], scalar=XS,
                        in1=rsb, op0=ALU.mult, op1=ALU.mult)

                # rs in token-major via PE transpose (rows of rsb^T are const)
                for t in range(2):
                    pst = psA.tile([P, 512], F32, tag="mm")
                    nc.tensor.transpose(pst[:, :P], rsb[:, t * P:(t + 1) * P],
                                        ident)
                    nc.scalar.copy(rstok[:, t:t + 1], pst[:, 0:1])

                # logits (token-major, fp32): (X'^T_chunk^T @ Wr^T) * rs + bias
                for t in range(2):
                    psr = psD.tile([P, E], F32, tag="aux")
                    for k in range(KD):
                        nc.tensor.matmul(
                            psr, xpT[:, k, t * P:(t + 1) * P], wr[:, k, :],
                            start=(k == 0), stop=(k == KD - 1))
                    nc.vector.scalar_tensor_tensor(
                        out=lg[:, t, :], in0=psr, scalar=rstok[:, t:t + 1],
                        in1=ebias_b, op0=ALU.mult, op1=ALU.add)

                # softmax over experts + top-2 renormalized combine weights
                for t in range(2):
                    lgt = lg[:, t, :]
                    m_s = aact.tile([P, 1], F32, tag="sm", name=f"m_{t}")
                    nc.vector.reduce_max(m_s, lgt, axis=AX)
                    negm = aact.tile([P, 1], F32, tag="sm2", name=f"nm_{t}")
                    nc.vector.tensor_scalar_mul(negm, m_s, -1.0)
                    e_s = aact.tile([P, E], F32, tag="sm3", name=f"e_{t}")
                    den = aact.tile([P, 1], F32, tag="sm4", name=f"d_{t}")
                    nc.scalar.activation(e_s, lgt, ACTF.Exp, bias=negm,
                                         accum_out=den)
                    rcp = aact.tile([P, 1], F32, tag="sm5", name=f"r_{t}")
                    nc.vector.reciprocal(rcp, den)
                    rw = aact.tile([P, E], F32, tag="sm6", name=f"rw_{t}")
                    nc.vector.tensor_scalar_mul(rw, e_s, rcp)
                    m1 = aact.tile([P, 1], F32, tag="sm7", name=f"m1_{t}")
                    nc.vector.reduce_max(m1, rw, axis=AX)
                    mask1 = aact.tile([P, E], F32, tag="sm8", name=f"k1_{t}")
                    nc.vector.tensor_scalar(mask1, rw, m1, None, ALU.is_ge)
                    rw2 = aact.tile([P, E], F32, tag="sm9", name=f"rw2_{t}")
                    nc.vector.scalar_tensor_tensor(
                        out=rw2, in0=mask1, scalar=-10.0, in1=rw,
                        op0=ALU.mult, op1=ALU.add)
                    m2 = aact.tile([P, 1], F32, tag="sm10", name=f"m2_{t}")
                    nc.vector.reduce_max(m2, rw2, axis=AX)
                    masktop = aact.tile([P, E], F32, tag="sm11", name=f"kt_{t}")
                    nc.vector.tensor_scalar(masktop, rw, m2, None, ALU.is_ge)
                    er = aact.tile([P, E], F32, tag="sm12", name=f"er_{t}")
                    nc.scalar.activation(er, rw, ACTF.Exp)
                    erm = aact.tile([P, E], F32, tag="sm13", name=f"em_{t}")
                    nc.vector.tensor_mul(out=erm, in0=er, in1=masktop)
                    den2 = aact.tile([P, 1], F32, tag="sm14", name=f"d2_{t}")
                    nc.vector.reduce_sum(den2, erm, axis=AX)
                    rcp2 = aact.tile([P, 1], F32, tag="sm15", name=f"r2_{t}")
                    nc.vector.reciprocal(rcp2, den2)
                    nc.vector.tensor_scalar_mul(comb[:, t, :], erm, rcp2)

                # combine^T [E, TQ] via PE transpose, then row-broadcast
                for t in range(2):
                    pst = psA.tile([P, 512], F32, tag="mm")
                    nc.tensor.transpose(pst[:E, :P], comb[:, t, :], ident)
                    nc.scalar.copy(cT[:, t * P:(t + 1) * P], pst[:E, :P])
                with tc.tile_pool(name="dbounce", bufs=1, space="DRAM") as dbp:
                    cdram = dbp.tile([E, TQ], F32)
                    nc.sync.dma_start(cdram, cT)
                    for e in range(E):
                        nc.sync.dma_start(
                            cb[:, e, :],
                            cdram[e:e + 1, :].to_broadcast([P, TQ]))

            # ---------------- shared expert + 8 routed experts ----------------
            with (
                tc.tile_pool(name="wmoe", bufs=2) as wmoe,
                tc.tile_pool(name="mact", bufs=2) as mact,
                tc.tile_pool(name="psM", bufs=5, space="PSUM") as psM,
                tc.tile_pool(name="psO", bufs=3, space="PSUM") as psO,
            ):
                # fp8 scales: psa = WS*XS*h1, hp stores HS*h (and combine w
                # for routed experts), pso = WS*HS*out.
                S1 = 1.0 / (WS * XS)
                SH = HS / (WS * XS)
                SO = 1.0 / (WS * HS)
                for u in range(E + 1):
                    if u == 0:
                        w1, w3, w2 = w1_0, w3_0, w2_0
                    else:
                        w1 = wmoe.tile([P, KD, F], F8, tag="w1")
                        nc.sync.dma_start(w1, _r(din["ew1"][u - 1], F))
                        w3 = wmoe.tile([P, KD, F], F8, tag="w3")
                        nc.sync.dma_start(w3, _r(din["ew3"][u - 1], F))
                        w2 = wmoe.tile([P, KF, D], F8, tag="w2")
                        nc.sync.dma_start(w2, _r(din["ew2"][u - 1], D))

                    hp = mact.tile([P, KF, TQ], F8, tag="hp")
                    for m in range(KF):
                        psa = psM.tile([P, TQ], F32, tag="ab")
                        for k in range(0, KD, 2):
                            nc.tensor.matmul(
                                psa, w1[:, k:k + 2, m * P:(m + 1) * P],
                                nx2[:, k:k + 2, :],
                                start=(k == 0), stop=(k == KD - 2),
                                perf_mode=DR)
                        sa = mact.tile([P, TQ], BF16, tag="sa")
                        nc.scalar.activation(sa, psa, ACTF.Silu, scale=S1)
                        psb2 = psM.tile([P, TQ], F32, tag="ab")
                        for k in range(0, KD, 2):
                            nc.tensor.matmul(
                                psb2, w3[:, k:k + 2, m * P:(m + 1) * P],
                                nx2[:, k:k + 2, :],
                                start=(k == 0), stop=(k == KD - 2),
                                perf_mode=DR)
                        if u == 0:
                            nc.vector.scalar_tensor_tensor(
                                out=hp[:, m, :], in0=psb2, scalar=SH,
                                in1=sa, op0=ALU.mult, op1=ALU.mult)
                        else:
                            tmp = mact.tile([P, TQ], BF16, tag="tmp")
                            nc.vector.tensor_mul(out=tmp, in0=sa,
                                                 in1=cb[:, u - 1, :])
                            nc.vector.scalar_tensor_tensor(
                                out=hp[:, m, :], in0=psb2, scalar=SH,
                                in1=tmp, op0=ALU.mult, op1=ALU.mult)
                    for m in range(KD):
                        pso = psO.tile([P, TQ], F32, tag="eo")
                        for k in range(0, KF, 2):
                            nc.tensor.matmul(
                                pso, w2[:, k:k + 2, m * P:(m + 1) * P],
                                hp[:, k:k + 2, :],
                                start=(k == 0), stop=(k == KF - 2),
                                perf_mode=DR)
                        if u == 0:
                            nc.vector.tensor_scalar_mul(spec[:, m, :], pso, SO)
                        elif u < E:
                            nc.vector.scalar_tensor_tensor(
                                out=spec[:, m, :], in0=pso, scalar=SO,
                                in1=spec[:, m, :], op0=ALU.mult, op1=ALU.add)
                        else:
                            nc.vector.scalar_tensor_tensor(
                                out=spec[:, m, :], in0=pso, scalar=SO,
                                in1=spec[:, m, :], op0=ALU.mult, op1=ALU.add)
                            nc.vector.tensor_add(out=outT[:, m, :],
                                                 in0=spec[:, m, :],
                                                 in1=xpT[:, m, :])
                            nc.sync.dma_start(
                                _r(outt[:], TQ)[:, m, :], outT[:, m, :])

    nc.finalize()
    return nc


def _prep_inputs(inputs):
    bf = ml_dtypes.bfloat16
    X = np.asarray(inputs["X"], np.float32)
    g1 = np.asarray(inputs["g1"], np.float32)
    rs1 = 1.0 / np.sqrt(np.mean(X * X, axis=-1, keepdims=True) + EPS)
    nx = X * rs1 * g1                      # [2, S, D] fp32
    nxT = np.ascontiguousarray(np.transpose(nx, (0, 2, 1))).astype(bf)
    XT = np.ascontiguousarray(np.transpose(X, (0, 2, 1)))

    def pm(a):
        """[C*P, N] -> partition-major [P, C*N]."""
        cp, n = a.shape
        return np.ascontiguousarray(
            a.reshape(cp // P, P, n).swapaxes(0, 1).reshape(P, -1))

    def t2(a):
        return pm(np.ascontiguousarray(np.asarray(a, np.float32).T))

    def t3(a):
        a = np.transpose(np.asarray(a, np.float32), (0, 2, 1))
        return np.stack([pm(np.ascontiguousarray(a[e])) for e in range(E)])

    f8 = ml_dtypes.float8_e4m3

    def q8(a):
        """fp8e4 quantize with the WS weight scale (clip to TRN max 240)."""
        return np.clip(a * WS, -240.0, 240.0).astype(f8)

    shared = {
        "wq": t2(inputs["Wq"]).astype(bf),
        "wkc": t2(inputs["Wkc"]).astype(bf),
        "wvc": t2(inputs["Wvc"]).astype(bf),
        "wo": t2(inputs["Wo"]).astype(bf),
        "wr": t2(inputs["Wr"]).astype(np.float32),
        "ebias": np.asarray(inputs["expert_bias"],
                            np.float32).reshape(1, E),
        "sw1": q8(t2(inputs["sW1"])),
        "sw3": q8(t2(inputs["sW3"])),
        "sw2": q8(t2(inputs["sW2"])),
        "ew1": q8(t3(inputs["eW1"])),
        "ew3": q8(t3(inputs["eW3"])),
        "ew2": q8(t3(inputs["eW2"])),
    }
    in_maps = []
    for c in range(8):
        b, q0 = c // 4, (c % 4) * TQ
        m = dict(shared)

        def pm(a):
            cp, n = a.shape
            return np.ascontiguousarray(
                a.reshape(cp // P, P, n).swapaxes(0, 1).reshape(P, -1))

        m["nxt"] = pm(nxT[b])
        m["nxq"] = pm(np.ascontiguousarray(nxT[b][:, q0:q0 + TQ]))
        m["xt"] = pm(np.ascontiguousarray(XT[b][:, q0:q0 + TQ]))
        in_maps.append(m)
    return in_maps


def run_on_device(inputs, trace=False):
    if "nc" not in _CACHE:
        _CACHE["nc"] = build_program()
    nc = _CACHE["nc"]
    in_maps = _prep_inputs(inputs)
    res = run_bass_kernel_spmd(nc, in_maps, core_ids=list(range(8)),
                               trace=trace)
    out = np.empty((2, S, D), np.float32)
    for c in range(8):
        b, q0 = c // 4, (c % 4) * TQ
        ot = res.results[c]["outt"].reshape(P, KD, TQ).swapaxes(0, 1)
        out[b, q0:q0 + TQ, :] = ot.reshape(D, TQ).T
    return out, res


def kernel(**inputs):
    out, _ = run_on_device(inputs, trace=False)
    return out



# revision 18
# speedup vs baseline: 1.4022x; 1.0443x over previous
"""DeepSeek layer (MLA attention + shared/routed MoE) on 8 TRN2 NeuronCores.

Data-parallel over tokens: core c handles batch c//4, tokens [(c%4)*256, ...).
Activations live feature-major [feature, token] on device; host pre-transposes
weights (bf16) and precomputes the first rmsnorm (depends only on input X).
Router logits are computed in fp32 so top-2 expert selection matches the
reference; expert matmuls run in bf16.
"""

import numpy as np
import ml_dtypes

import concourse.bass as bass
import concourse.tile as tile
from concourse import bacc, mybir
from concourse.bass_utils import run_bass_kernel_spmd
from concourse.masks import make_identity

BF16 = mybir.dt.bfloat16
F32 = mybir.dt.float32
F8 = mybir.dt.float8e4
AX = mybir.AxisListType.X
ALU = mybir.AluOpType
ACTF = mybir.ActivationFunctionType
DR = mybir.MatmulPerfMode.DoubleRow

# fp8 scale factors: weights x64, activations x16, hidden x8
WS = 64.0
XS = 16.0
HS = 8.0

P = 128
D = 1024
KD = D // P          # 8 feature chunks
S = 1024             # keys per batch
TQ = 256             # query tokens per core
H = 4
DC = 256             # compressed kv dim == dk
F = 1024
KF = F // P
E = 8
EPS = 1e-6
SCALE = 1.0 / 16.0   # 1/sqrt(dk)

_CACHE = {}


def _r(ap, n=None):
    """Host-permuted DRAM [P, C*N] -> [P, C, N] view (contiguous)."""
    c = ap.shape[-1]
    n = n if n is not None else c // KD
    return ap.rearrange("p (k n) -> p k n", n=n)


def build_program():
    nc = bacc.Bacc(None)

    # All tensors are host-permuted to partition-major [P, chunks*N] so each
    # DMA is one contiguous segment per partition (descriptor-rate matters).
    din = {}
    F32R = mybir.dt.float32r
    for name, shape, dt in [
        ("nxt", [P, KD * S], F8),
        ("nxq", [P, KD * TQ], F8),
        ("xt", [P, KD * TQ], F32),
        ("wq", [P, KD * D], F8),
        ("wkc", [P, KD * DC], F8),
        ("wvc", [P, KD * DC], F8),
        ("wo", [P, KD * D], F8),
        ("wr", [P, KD * E], F32),
        ("ebias", [1, E], F32),
        ("sw1", [P, KD * F], F8),
        ("sw3", [P, KD * F], F8),
        ("sw2", [P, KF * D], F8),
        ("ew1", [E, P, KD * F], F8),
        ("ew3", [E, P, KD * F], F8),
        ("ew2", [E, P, KF * D], F8),
    ]:
        din[name] = nc.dram_tensor(name, shape, dt, kind="ExternalInput")
    outt = nc.dram_tensor("outt", [P, KD * TQ], F32, kind="ExternalOutput")

    with tile.TileContext(nc) as tc:
        with (
            tc.tile_pool(name="const", bufs=1) as const,
            tc.tile_pool(name="persist", bufs=1) as persist,
        ):
            ones_f8 = const.tile([P, 1], F8)
            nc.vector.memset(ones_f8, 1.0)
            ones_cf = const.tile([P, 1], F32)
            nc.vector.memset(ones_cf, 1.0)
            ones_row = const.tile([1, P], F32)
            nc.vector.memset(ones_row, 1.0)
            eps1 = const.tile([1, 1], F32)
            nc.vector.memset(eps1, EPS)
            ident = const.tile([P, P], F32)
            make_identity(nc, ident)
            ebias_b = const.tile([P, E], F32)
            nc.sync.dma_start(ebias_b, din["ebias"][:].to_broadcast([P, E]))
            # warm all three activation tables (Exp/Sqrt/Silu) up front so
            # the 1.3us table loads overlap the input DMA instead of the
            # serial rmsnorm/router path
            warm = const.tile([1, 4], F32)
            nc.scalar.activation(warm[0:1, 0:1], ones_cf[0:1, 0:1], ACTF.Exp)
            nc.scalar.activation(warm[0:1, 1:2], ones_cf[0:1, 0:1], ACTF.Sqrt)
            nc.scalar.activation(warm[0:1, 2:3], ones_cf[0:1, 0:1],
                                 ACTF.Silu)

            xpT = persist.tile([P, KD, TQ], F32)      # X' = X + attn out
            nx2 = persist.tile([P, KD, TQ], F8)       # rmsnorm2(X')*XS fp8
            cb = persist.tile([P, E, TQ], F32)        # combine weights bcast
            spec = persist.tile([P, KD, TQ], F32)     # shared+experts accum
            outT = persist.tile([P, KD, TQ], F32)

            # shared-expert weights, prefetched during attention so the MoE
            # phase starts without a DMA stall (DMAs issued after the
            # attention inputs below)
            w1_0 = persist.tile([P, KD, F], F8)
            w3_0 = persist.tile([P, KD, F], F8)
            w2_0 = persist.tile([P, KF, D], F8)

            # ---------------- attention ----------------
            with (
                tc.tile_pool(name="ainp", bufs=1) as ainp,
                tc.tile_pool(name="awts", bufs=1) as awts,
                tc.tile_pool(name="aact", bufs=1) as aact,
                tc.tile_pool(name="psA", bufs=3, space="PSUM") as psA,
                tc.tile_pool(name="psD", bufs=2, space="PSUM") as psD,
            ):
                # DMA issue order: Q-path operands first so PE starts early
                nxq = ainp.tile([P, KD, TQ], F8)
                nc.sync.dma_start(nxq, _r(din["nxq"][:], TQ))
                wq = awts.tile([P, KD, D], F8)
                nc.sync.dma_start(wq, _r(din["wq"][:], D))
                wkc = awts.tile([P, KD, DC], F8)
                nc.sync.dma_start(wkc, _r(din["wkc"][:], DC))
                nxt = ainp.tile([P, KD, S], F8)
                nc.sync.dma_start(nxt, _r(din["nxt"][:], S))
                wvc = awts.tile([P, KD, DC], F8)
                nc.sync.dma_start(wvc, _r(din["wvc"][:], DC))
                xt = ainp.tile([P, KD, TQ], F32)
                nc.sync.dma_start(xt, _r(din["xt"][:], TQ))
                wo = awts.tile([P, KD, D], F8)
                nc.sync.dma_start(wo, _r(din["wo"][:], D))
                wrr = awts.tile([P, KD, E], F32)
                nc.sync.dma_start(wrr, _r(din["wr"][:], E))

                qT = aact.tile([P, KD, TQ], F8)
                kcT = aact.tile([P, 2, S], F8)
                vc = aact.tile([P, KD, DC], F8)
                probs = aact.tile([P, KD, H, TQ], F8)
                rcpd = aact.tile([1, H, TQ], F32)
                rcpb = aact.tile([P, H, TQ], F32)
                oT = aact.tile([P, KD, TQ], F8)

                # Q^T [d, tq] = Wq @ nxq; psum = WS*XS*q, copy back at XS*q
                for m in range(KD):
                    ps = psA.tile([P, 512], F32, tag="mm")
                    for k in range(0, KD, 2):
                        nc.tensor.matmul(
                            ps[:, :TQ], wq[:, k:k + 2, m * P:(m + 1) * P],
                            nxq[:, k:k + 2, :], start=(k == 0),
                            stop=(k == KD - 2), perf_mode=DR)
                    nc.scalar.activation(qT[:, m, :], ps[:, :TQ], ACTF.Copy,
                                         scale=1.0 / WS)

                # Kc^T [dc, S] = Wkc @ nxt (kept at XS*kc)
                for m in range(2):
                    for n2 in range(2):
                        ps = psA.tile([P, 512], F32, tag="mm")
                        for k in range(0, KD, 2):
                            nc.tensor.matmul(
                                ps, wkc[:, k:k + 2, m * P:(m + 1) * P],
                                nxt[:, k:k + 2, n2 * 512:(n2 + 1) * 512],
                                start=(k == 0), stop=(k == KD - 2),
                                perf_mode=DR)
                        nc.scalar.activation(
                            kcT[:, m, n2 * 512:(n2 + 1) * 512], ps, ACTF.Copy,
                            scale=1.0 / WS)

                # Vc [S, dc] = nxt^T @ Wvc^T  (keys-major, XS*vc)
                for kc in range(KD):
                    ps = psA.tile([P, 512], F32, tag="mm")
                    for k in range(0, KD, 2):
                        nc.tensor.matmul(
                            ps[:, :DC], nxt[:, k:k + 2, kc * P:(kc + 1) * P],
                            wvc[:, k:k + 2, :], start=(k == 0),
                            stop=(k == KD - 2), perf_mode=DR)
                    nc.scalar.activation(vc[:, kc, :], ps[:, :DC], ACTF.Copy,
                                         scale=1.0 / WS)

                # scores^T + exp (keys-major, no max-sub needed); psum holds
                # XS*XS*s so fold 1/XS^2 into the exp scale
                for h in range(H):
                    for kc in range(KD):
                        ps = psA.tile([P, 512], F32, tag="mm")
                        nc.tensor.matmul(
                            ps[:, :TQ], kcT[:, 0:2, kc * P:(kc + 1) * P],
                            qT[:, 2 * h:2 * h + 2, :], start=True, stop=True,
                            perf_mode=DR)
                        nc.scalar.activation(
                            probs[:, kc, h, :], ps[:, :TQ], ACTF.Exp,
                            scale=SCALE / (XS * XS))

                # prefetch shared-expert weights now: attention inputs are
                # in flight, MoE phase is ~50us away
                nc.sync.dma_start(w1_0, _r(din["sw1"][:], F))
                nc.sync.dma_start(w3_0, _r(din["sw3"][:], F))
                nc.sync.dma_start(w2_0, _r(din["sw2"][:], D))

                # softmax denominators + reciprocal + broadcast
                for h in range(H):
                    psd = psD.tile([1, TQ], F32, tag="aux")
                    for kc in range(KD):
                        nc.tensor.matmul(
                            psd, ones_f8[:, 0:1], probs[:, kc, h, :],
                            start=(kc == 0), stop=(kc == KD - 1))
                    nc.vector.reciprocal(rcpd[0:1, h, :], psd)
                    psb = psD.tile([P, TQ], F32, tag="aux")
                    nc.tensor.matmul(psb, ones_row, rcpd[0:1, h, :],
                                     start=True, stop=True)
                    nc.scalar.copy(rcpb[:, h, :], psb)

                # out_h^T = Vc^T @ probs^T, normalized per token (XS*out)
                for h in range(H):
                    for m in range(2):
                        ps = psA.tile([P, 512], F32, tag="mm")
                        for kc in range(0, KD, 2):
                            nc.tensor.matmul(
                                ps[:, :TQ],
                                vc[:, kc:kc + 2, m * P:(m + 1) * P],
                                probs[:, kc:kc + 2, h, :],
                                start=(kc == 0), stop=(kc == KD - 2),
                                perf_mode=DR)
                        nc.vector.tensor_mul(
                            out=oT[:, 2 * h + m, :], in0=ps[:, :TQ],
                            in1=rcpb[:, h, :])

                # attn proj + residual: X' = Wo @ O / (WS*XS) + X
                for m in range(KD):
                    ps = psA.tile([P, 512], F32, tag="mm")
                    for k in range(0, KD, 2):
                        nc.tensor.matmul(
                            ps[:, :TQ], wo[:, k:k + 2, m * P:(m + 1) * P],
                            oT[:, k:k + 2, :], start=(k == 0),
                            stop=(k == KD - 2), perf_mode=DR)
                    nc.vector.scalar_tensor_tensor(
                        out=xpT[:, m, :], in0=ps[:, :TQ],
                        scalar=1.0 / (WS * XS), in1=xt[:, m, :],
                        op0=ALU.mult, op1=ALU.add)

                # ---------------- rmsnorm2 + router ----------------
                sq = aact.tile([P, KD, TQ], F32)
                rs = aact.tile([1, TQ], F32)
                sd = aact.tile([1, TQ], F32)
                rsb = aact.tile([P, TQ], F32)
                rstok = aact.tile([P, 2], F32)
                lg = aact.tile([P, 2, E], F32)
                comb = aact.tile([P, 2, E], F32)
                cT = aact.tile([E, TQ], F32)

                for m in range(KD):
                    nc.scalar.square(sq[:, m, :], xpT[:, m, :])
                psq = psD.tile([1, TQ], F32, tag="aux")
                for k in range(KD):
                    nc.tensor.matmul(psq, ones_cf[:, 0:1], sq[:, k, :],
                                     start=(k == 0), stop=(k == KD - 1))
                nc.scalar.activation(sd[0:1, :], psq, ACTF.Sqrt,
                                     bias=eps1[0:1, :], scale=1.0 / D)
                nc.vector.reciprocal(rs[0:1, :], sd[0:1, :])
                psb = psD.tile([P, TQ], F32, tag="aux")
                nc.tensor.matmul(psb, ones_row, rs[0:1, :], start=True, stop=True)
                nc.scalar.copy(rsb, psb)
                for m in range(KD):
                    nc.vector.scalar_tensor_tensor(
                        out=nx2[:, m, :], in0=xpT[:, m, :], scalar=XS,
                        in1=rsb, op0=ALU.mult, op1=ALU.mult)

                # rs in token-major via PE transpose (rows of rsb^T are const)
                for t in range(2):
                    pst = psA.tile([P, 512], F32, tag="mm")
                    nc.tensor.transpose(pst[:, :P], rsb[:, t * P:(t + 1) * P],
                                        ident)
                    nc.scalar.copy(rstok[:, t:t + 1], pst[:, 0:1])

                # logits^T [E, TQ] with Wr stationary (8-col LDWEIGHTS is
                # nearly free), then PE-transpose to token-major
                psrT = psD.tile([E, TQ], F32, tag="auxr")
                for k in range(KD):
                    nc.tensor.matmul(
                        psrT, wrr[:, k, :], xpT[:, k, :],
                        start=(k == 0), stop=(k == KD - 1))
                lgTs = aact.tile([E, TQ], F32)
                nc.scalar.copy(lgTs, psrT)
                for t in range(2):
                    pst2 = psA.tile([P, 512], F32, tag="mm")
                    nc.tensor.transpose(pst2[:, :E],
                                        lgTs[:, t * P:(t + 1) * P],
                                        ident[:E, :E])
                    nc.vector.scalar_tensor_tensor(
                        out=lg[:, t, :], in0=pst2[:, :E],
                        scalar=rstok[:, t:t + 1],
                        in1=ebias_b, op0=ALU.mult, op1=ALU.add)

                # softmax over experts + top-2 renormalized combine weights
                for t in range(2):
                    lgt = lg[:, t, :]
                    m_s = aact.tile([P, 1], F32, tag="sm", name=f"m_{t}")
                    nc.vector.reduce_max(m_s, lgt, axis=AX)
                    negm = aact.tile([P, 1], F32, tag="sm2", name=f"nm_{t}")
                    nc.vector.tensor_scalar_mul(negm, m_s, -1.0)
                    e_s = aact.tile([P, E], F32, tag="sm3", name=f"e_{t}")
                    den = aact.tile([P, 1], F32, tag="sm4", name=f"d_{t}")
                    nc.scalar.activation(e_s, lgt, ACTF.Exp, bias=negm,
                                         accum_out=den)
                    rcp = aact.tile([P, 1], F32, tag="sm5", name=f"r_{t}")
                    nc.vector.reciprocal(rcp, den)
                    rw = aact.tile([P, E], F32, tag="sm6", name=f"rw_{t}")
                    nc.vector.tensor_scalar_mul(rw, e_s, rcp)
                    m1 = aact.tile([P, 1], F32, tag="sm7", name=f"m1_{t}")
                    nc.vector.reduce_max(m1, rw, axis=AX)
                    mask1 = aact.tile([P, E], F32, tag="sm8", name=f"k1_{t}")
                    nc.vector.tensor_scalar(mask1, rw, m1, None, ALU.is_ge)
                    rw2 = aact.tile([P, E], F32, tag="sm9", name=f"rw2_{t}")
                    nc.vector.scalar_tensor_tensor(
                        out=rw2, in0=mask1, scalar=-10.0, in1=rw,
                        op0=ALU.mult, op1=ALU.add)
                    m2 = aact.tile([P, 1], F32, tag="sm10", name=f"m2_{t}")
                    nc.vector.reduce_max(m2, rw2, axis=AX)
                    masktop = aact.tile([P, E], F32, tag="sm11", name=f"kt_{t}")
                    nc.vector.tensor_scalar(masktop, rw, m2, None, ALU.is_ge)
                    er = aact.tile([P, E], F32, tag="sm12", name=f"er_{t}")
                    nc.scalar.activation(er, rw, ACTF.Exp)
                    erm = aact.tile([P, E], F32, tag="sm13", name=f"em_{t}")
                    nc.vector.tensor_mul(out=erm, in0=er, in1=masktop)
                    den2 = aact.tile([P, 1], F32, tag="sm14", name=f"d2_{t}")
                    nc.vector.reduce_sum(den2, erm, axis=AX)
                    rcp2 = aact.tile([P, 1], F32, tag="sm15", name=f"r2_{t}")
                    nc.vector.reciprocal(rcp2, den2)
                    nc.vector.tensor_scalar_mul(comb[:, t, :], erm, rcp2)

                # combine^T [E, TQ] via PE transpose, then row-broadcast
                for t in range(2):
                    pst = psA.tile([P, 512], F32, tag="mm")
                    nc.tensor.transpose(pst[:E, :P], comb[:, t, :], ident)
                    nc.scalar.copy(cT[:, t * P:(t + 1) * P], pst[:E, :P])
                with tc.tile_pool(name="dbounce", bufs=1, space="DRAM") as dbp:
                    cdram = dbp.tile([E, TQ], F32)
                    nc.sync.dma_start(cdram, cT)
                    for e in range(E):
                        nc.sync.dma_start(
                            cb[:, e, :],
                            cdram[e:e + 1, :].to_broadcast([P, TQ]))

            # ---------------- shared expert + 8 routed experts ----------------
            with (
                tc.tile_pool(name="wmoe", bufs=2) as wmoe,
                tc.tile_pool(name="mact", bufs=2) as mact,
                tc.tile_pool(name="psM", bufs=5, space="PSUM") as psM,
                tc.tile_pool(name="psO", bufs=3, space="PSUM") as psO,
            ):
                # fp8 scales: psa = WS*XS*h1, hp stores HS*h (and combine w
                # for routed experts), pso = WS*HS*out.
                S1 = 1.0 / (WS * XS)
                SH = HS / (WS * XS)
                SO = 1.0 / (WS * HS)
                for u in range(E + 1):
                    if u == 0:
                        w1, w3, w2 = w1_0, w3_0, w2_0
                    else:
                        w1 = wmoe.tile([P, KD, F], F8, tag="w1")
                        nc.sync.dma_start(w1, _r(din["ew1"][u - 1], F))
                        w3 = wmoe.tile([P, KD, F], F8, tag="w3")
                        nc.sync.dma_start(w3, _r(din["ew3"][u - 1], F))
                        w2 = wmoe.tile([P, KF, D], F8, tag="w2")
                        nc.sync.dma_start(w2, _r(din["ew2"][u - 1], D))

                    hp = mact.tile([P, KF, TQ], F8, tag="hp")
                    for m in range(KF):
                        psa = psM.tile([P, TQ], F32, tag="ab")
                        for k in range(0, KD, 2):
                            nc.tensor.matmul(
                                psa, w1[:, k:k + 2, m * P:(m + 1) * P],
                                nx2[:, k:k + 2, :],
                                start=(k == 0), stop=(k == KD - 2),
                                perf_mode=DR)
                        sa = mact.tile([P, TQ], BF16, tag="sa")
                        nc.scalar.activation(sa, psa, ACTF.Silu, scale=S1)
                        psb2 = psM.tile([P, TQ], F32, tag="ab")
                        for k in range(0, KD, 2):
                            nc.tensor.matmul(
                                psb2, w3[:, k:k + 2, m * P:(m + 1) * P],
                                nx2[:, k:k + 2, :],
                                start=(k == 0), stop=(k == KD - 2),
                                perf_mode=DR)
                        if u == 0:
                            nc.vector.scalar_tensor_tensor(
                                out=hp[:, m, :], in0=psb2, scalar=SH,
                                in1=sa, op0=ALU.mult, op1=ALU.mult)
                        else:
                            tmp = mact.tile([P, TQ], BF16, tag="tmp")
                            nc.vector.tensor_mul(out=tmp, in0=sa,
                                                 in1=cb[:, u - 1, :])
                            nc.vector.scalar_tensor_tensor(
                                out=hp[:, m, :], in0=psb2, scalar=SH,
                                in1=tmp, op0=ALU.mult, op1=ALU.mult)
                    for m in range(KD):
                        pso = psO.tile([P, TQ], F32, tag="eo")
                        for k in range(0, KF, 2):
                            nc.tensor.matmul(
                                pso, w2[:, k:k + 2, m * P:(m + 1) * P],
                                hp[:, k:k + 2, :],
                                start=(k == 0), stop=(k == KF - 2),
                                perf_mode=DR)
                        if u == 0:
                            nc.vector.tensor_scalar_mul(spec[:, m, :], pso, SO)
                        elif u < E:
                            nc.vector.scalar_tensor_tensor(
                                out=spec[:, m, :], in0=pso, scalar=SO,
                                in1=spec[:, m, :], op0=ALU.mult, op1=ALU.add)
                        else:
                            nc.vector.scalar_tensor_tensor(
                                out=spec[:, m, :], in0=pso, scalar=SO,
                                in1=spec[:, m, :], op0=ALU.mult, op1=ALU.add)
                            nc.vector.tensor_add(out=outT[:, m, :],
                                                 in0=spec[:, m, :],
                                                 in1=xpT[:, m, :])
                            nc.sync.dma_start(
                                _r(outt[:], TQ)[:, m, :], outT[:, m, :])

    nc.finalize()
    return nc


def _prep_inputs(inputs):
    bf = ml_dtypes.bfloat16
    f8 = ml_dtypes.float8_e4m3
    X = np.asarray(inputs["X"], np.float32)
    g1 = np.asarray(inputs["g1"], np.float32)
    rs1 = 1.0 / np.sqrt(np.mean(X * X, axis=-1, keepdims=True) + EPS)
    nx = X * rs1 * g1                      # [2, S, D] fp32
    nxT = np.clip(np.ascontiguousarray(np.transpose(nx, (0, 2, 1))) * XS,
                  -240.0, 240.0).astype(f8)
    XT = np.ascontiguousarray(np.transpose(X, (0, 2, 1)))

    def pm(a):
        """[C*P, N] -> partition-major [P, C*N]."""
        cp, n = a.shape
        return np.ascontiguousarray(
            a.reshape(cp // P, P, n).swapaxes(0, 1).reshape(P, -1))

    def t2(a):
        return pm(np.ascontiguousarray(np.asarray(a, np.float32).T))

    def t3(a):
        a = np.transpose(np.asarray(a, np.float32), (0, 2, 1))
        return np.stack([pm(np.ascontiguousarray(a[e])) for e in range(E)])

    f8 = ml_dtypes.float8_e4m3

    def q8(a):
        """fp8e4 quantize with the WS weight scale (clip to TRN max 240)."""
        return np.clip(a * WS, -240.0, 240.0).astype(f8)

    shared = {
        "wq": q8(t2(inputs["Wq"])),
        "wkc": q8(t2(inputs["Wkc"])),
        "wvc": q8(t2(inputs["Wvc"])),
        "wo": q8(t2(inputs["Wo"])),
        "wr": t2(inputs["Wr"]).astype(np.float32),
        "ebias": np.asarray(inputs["expert_bias"],
                            np.float32).reshape(1, E),
        "sw1": q8(t2(inputs["sW1"])),
        "sw3": q8(t2(inputs["sW3"])),
        "sw2": q8(t2(inputs["sW2"])),
        "ew1": q8(t3(inputs["eW1"])),
        "ew3": q8(t3(inputs["eW3"])),
        "ew2": q8(t3(inputs["eW2"])),
    }
    in_maps = []
    for c in range(8):
        b, q0 = c // 4, (c % 4) * TQ
        m = dict(shared)

        def pm(a):
            cp, n = a.shape
            return np.ascontiguousarray(
                a.reshape(cp // P, P, n).swapaxes(0, 1).reshape(P, -1))

        m["nxt"] = pm(nxT[b])
        m["nxq"] = pm(np.ascontiguousarray(nxT[b][:, q0:q0 + TQ]))
        m["xt"] = pm(np.ascontiguousarray(XT[b][:, q0:q0 + TQ]))
        in_maps.append(m)
    return in_maps


def run_on_device(inputs, trace=False):
    if "nc" not in _CACHE:
        _CACHE["nc"] = build_program()
    nc = _CACHE["nc"]
    in_maps = _prep_inputs(inputs)
    res = run_bass_kernel_spmd(nc, in_maps, core_ids=list(range(8)),
                               trace=trace)
    out = np.empty((2, S, D), np.float32)
    for c in range(8):
        b, q0 = c // 4, (c % 4) * TQ
        ot = res.results[c]["outt"].reshape(P, KD, TQ).swapaxes(0, 1)
        out[b, q0:q0 + TQ, :] = ot.reshape(D, TQ).T
    return out, res


def kernel(**inputs):
    out, _ = run_on_device(inputs, trace=False)
    return out



# revision 27
# speedup vs baseline: 1.5605x; 1.1129x over previous
"""DeepSeek layer (MLA attention + shared/routed MoE) on 8 TRN2 NeuronCores.

Data-parallel over tokens: core c handles batch c//4, tokens [(c%4)*256, ...).
Activations live feature-major [feature, token] on device; host pre-transposes
weights (bf16) and precomputes the first rmsnorm (depends only on input X).
Router logits are computed in fp32 so top-2 expert selection matches the
reference; expert matmuls run in bf16.
"""

import numpy as np
import ml_dtypes

import concourse.bass as bass
import concourse.tile as tile
from concourse import bacc, mybir
from concourse.bass_utils import run_bass_kernel_spmd
from concourse.masks import make_identity

BF16 = mybir.dt.bfloat16
F32 = mybir.dt.float32
F8 = mybir.dt.float8e4
AX = mybir.AxisListType.X
ALU = mybir.AluOpType
ACTF = mybir.ActivationFunctionType
DR = mybir.MatmulPerfMode.DoubleRow

# fp8 scale factors: weights x64, activations x16, hidden x8
WS = 64.0
XS = 16.0
HS = 8.0

P = 128
D = 1024
KD = D // P          # 8 feature chunks
S = 1024             # keys per batch
TQ = 256             # query tokens per core
H = 4
DC = 256             # compressed kv dim == dk
F = 1024
KF = F // P
E = 8
EPS = 1e-6
SCALE = 1.0 / 16.0   # 1/sqrt(dk)

_CACHE = {}


def _r(ap, n=None):
    """Host-permuted DRAM [P, C*N] -> [P, C, N] view (contiguous)."""
    c = ap.shape[-1]
    n = n if n is not None else c // KD
    return ap.rearrange("p (k n) -> p k n", n=n)


def build_program():
    nc = bacc.Bacc(None)

    # All tensors are host-permuted to partition-major [P, chunks*N] so each
    # DMA is one contiguous segment per partition (descriptor-rate matters).
    din = {}
    F32R = mybir.dt.float32r
    for name, shape, dt in [
        ("nxt", [P, KD * S], F8),
        ("nxq", [P, KD * TQ], F8),
        ("xt", [P, KD * TQ], F32),
        ("wq", [P, KD * D], F8),
        ("wkc", [P, KD * DC], F8),
        ("wvc", [P, KD * DC], F8),
        ("wo", [P, KD * D], F8),
        ("wr", [P, KD * E], F32),
        ("ebias", [1, E], F32),
        ("sw1", [P, KD * F], F8),
        ("sw3", [P, KD * F], F8),
        ("sw2", [P, KF * D], F8),
        ("ew1", [E, P, KD * F], F8),
        ("ew3", [E, P, KD * F], F8),
        ("ew2", [E, P, KF * D], F8),
    ]:
        din[name] = nc.dram_tensor(name, shape, dt, kind="ExternalInput")
    outt = nc.dram_tensor("outt", [P, KD * TQ], F32, kind="ExternalOutput")

    with tile.TileContext(nc) as tc:
        with (
            tc.tile_pool(name="const", bufs=1) as const,
            tc.tile_pool(name="persist", bufs=1) as persist,
        ):
            ones_f8 = const.tile([P, 1], F8)
            nc.vector.memset(ones_f8, 1.0)
            ones_cf = const.tile([P, 1], F32)
            nc.vector.memset(ones_cf, 1.0)
            ones_row = const.tile([1, P], F32)
            nc.vector.memset(ones_row, 1.0)
            eps1 = const.tile([1, 1], F32)
            nc.vector.memset(eps1, EPS)
            ident = const.tile([P, P], F32)
            make_identity(nc, ident)
            ebias_b = const.tile([P, E], F32)
            nc.sync.dma_start(ebias_b, din["ebias"][:].to_broadcast([P, E]))
            # warm all three activation tables (Exp/Sqrt/Silu) up front so
            # the 1.3us table loads overlap the input DMA instead of the
            # serial rmsnorm/router path
            warm = const.tile([1, 4], F32)
            nc.scalar.activation(warm[0:1, 0:1], ones_cf[0:1, 0:1], ACTF.Exp)
            nc.scalar.activation(warm[0:1, 1:2], ones_cf[0:1, 0:1], ACTF.Sqrt)
            nc.scalar.activation(warm[0:1, 2:3], ones_cf[0:1, 0:1],
                                 ACTF.Silu)

            xpT = persist.tile([P, KD, TQ], F32)      # X' = X + attn out
            nx2 = persist.tile([P, KD, TQ], F8)       # rmsnorm2(X')*XS fp8
            cb = persist.tile([P, E, TQ], F32)        # combine weights bcast
            spec = persist.tile([P, KD, TQ], F32)     # shared+experts accum
            outT = persist.tile([P, KD, TQ], F32)

            # shared-expert weights, prefetched during attention so the MoE
            # phase starts without a DMA stall (DMAs issued after the
            # attention inputs below)
            w1_0 = persist.tile([P, KD, F], F8)
            w3_0 = persist.tile([P, KD, F], F8)
            w2_0 = persist.tile([P, KF, D], F8)

            # ---------------- attention ----------------
            with (
                tc.tile_pool(name="ainp", bufs=1) as ainp,
                tc.tile_pool(name="awts", bufs=1) as awts,
                tc.tile_pool(name="aact", bufs=1) as aact,
                tc.tile_pool(name="psA", bufs=3, space="PSUM") as psA,
                tc.tile_pool(name="psD", bufs=2, space="PSUM") as psD,
                tc.tile_pool(name="psT", bufs=1, space="PSUM") as psT,
            ):
                # DMA issue order: Q-path operands first so PE starts early
                nxq = ainp.tile([P, KD, TQ], F8)
                nc.sync.dma_start(nxq, _r(din["nxq"][:], TQ))
                wq = awts.tile([P, KD, D], F8)
                nc.sync.dma_start(wq[:, :, :512], _r(din["wq"][:], D)[:, :, :512])
                nc.sync.dma_start(wq[:, :, 512:], _r(din["wq"][:], D)[:, :, 512:])
                wkc = awts.tile([P, KD, DC], F8)
                nc.sync.dma_start(wkc, _r(din["wkc"][:], DC))
                nxt = ainp.tile([P, KD, S], F8)
                nc.sync.dma_start(nxt[:, :, :512], _r(din["nxt"][:], S)[:, :, :512])
                nc.sync.dma_start(nxt[:, :, 512:], _r(din["nxt"][:], S)[:, :, 512:])
                wvc = awts.tile([P, KD, DC], F8)
                nc.sync.dma_start(wvc, _r(din["wvc"][:], DC))
                xt = ainp.tile([P, KD, TQ], F32)
                nc.sync.dma_start(xt, _r(din["xt"][:], TQ))
                wo = awts.tile([P, KD, D], F8)
                nc.sync.dma_start(wo, _r(din["wo"][:], D))
                wrr = awts.tile([P, KD, E], F32)
                nc.sync.dma_start(wrr, _r(din["wr"][:], E))

                qT = aact.tile([P, KD, TQ], F8)
                kcT = aact.tile([P, 2, S], F8)
                vc = aact.tile([P, KD, DC], F8)
                probs = aact.tile([P, KD, H, TQ], F8)
                rcpd = aact.tile([1, H, TQ], F32)
                rcpb = aact.tile([P, H, TQ], F32)
                oT = aact.tile([P, KD, TQ], F8)

                # Q^T [d, tq] = Wq @ nxq; psum = WS*XS*q, copy back at XS*q
                for m in range(KD):
                    ps = psA.tile([P, 512], F32, tag="mm")
                    for k in range(0, KD, 2):
                        nc.tensor.matmul(
                            ps[:, :TQ], wq[:, k:k + 2, m * P:(m + 1) * P],
                            nxq[:, k:k + 2, :], start=(k == 0),
                            stop=(k == KD - 2), perf_mode=DR)
                    nc.scalar.activation(qT[:, m, :], ps[:, :TQ], ACTF.Copy,
                                         scale=1.0 / WS)

                # Kc^T [dc, S] = Wkc @ nxt (kept at XS*kc)
                for m in range(2):
                    for n2 in range(2):
                        ps = psA.tile([P, 512], F32, tag="mm")
                        for k in range(0, KD, 2):
                            nc.tensor.matmul(
                                ps, wkc[:, k:k + 2, m * P:(m + 1) * P],
                                nxt[:, k:k + 2, n2 * 512:(n2 + 1) * 512],
                                start=(k == 0), stop=(k == KD - 2),
                                perf_mode=DR)
                        nc.scalar.activation(
                            kcT[:, m, n2 * 512:(n2 + 1) * 512], ps, ACTF.Copy,
                            scale=1.0 / WS)

                # Vc [S, dc] = nxt^T @ Wvc^T  (keys-major, XS*vc)
                for kc in range(KD):
                    ps = psA.tile([P, 512], F32, tag="mm")
                    for k in range(0, KD, 2):
                        nc.tensor.matmul(
                            ps[:, :DC], nxt[:, k:k + 2, kc * P:(kc + 1) * P],
                            wvc[:, k:k + 2, :], start=(k == 0),
                            stop=(k == KD - 2), perf_mode=DR)
                    nc.scalar.activation(vc[:, kc, :], ps[:, :DC], ACTF.Copy,
                                         scale=1.0 / WS)

                # scores^T + exp (keys-major, no max-sub needed); psum holds
                # XS*XS*s so fold 1/XS^2 into the exp scale
                for h in range(H):
                    for kc in range(KD):
                        ps = psA.tile([P, 512], F32, tag="mm")
                        nc.tensor.matmul(
                            ps[:, :TQ], kcT[:, 0:2, kc * P:(kc + 1) * P],
                            qT[:, 2 * h:2 * h + 2, :], start=True, stop=True,
                            perf_mode=DR)
                        nc.scalar.activation(
                            probs[:, kc, h, :], ps[:, :TQ], ACTF.Exp,
                            scale=SCALE / (XS * XS))

                # prefetch shared-expert weights now: attention inputs are
                # in flight, MoE phase is ~50us away
                nc.sync.dma_start(w1_0, _r(din["sw1"][:], F))
                nc.sync.dma_start(w3_0, _r(din["sw3"][:], F))
                nc.sync.dma_start(w2_0, _r(din["sw2"][:], D))

                # softmax denominators + reciprocal + broadcast
                for h in range(H):
                    psd = psD.tile([1, TQ], F32, tag="aux")
                    for kc in range(KD):
                        nc.tensor.matmul(
                            psd, ones_f8[:, 0:1], probs[:, kc, h, :],
                            start=(kc == 0), stop=(kc == KD - 1))
                    nc.vector.reciprocal(rcpd[0:1, h, :], psd)
                    psb = psD.tile([P, TQ], F32, tag="aux")
                    nc.tensor.matmul(psb, ones_row, rcpd[0:1, h, :],
                                     start=True, stop=True)
                    nc.scalar.copy(rcpb[:, h, :], psb)

                # out_h^T = Vc^T @ probs^T, normalized per token (XS*out)
                for h in range(H):
                    for m in range(2):
                        ps = psA.tile([P, 512], F32, tag="mm")
                        for kc in range(0, KD, 2):
                            nc.tensor.matmul(
                                ps[:, :TQ],
                                vc[:, kc:kc + 2, m * P:(m + 1) * P],
                                probs[:, kc:kc + 2, h, :],
                                start=(kc == 0), stop=(kc == KD - 2),
                                perf_mode=DR)
                        nc.vector.tensor_mul(
                            out=oT[:, 2 * h + m, :], in0=ps[:, :TQ],
                            in1=rcpb[:, h, :])

                # attn proj + residual: X' = Wo @ O / (WS*XS) + X
                for m in range(KD):
                    ps = psA.tile([P, 512], F32, tag="mm")
                    for k in range(0, KD, 2):
                        nc.tensor.matmul(
                            ps[:, :TQ], wo[:, k:k + 2, m * P:(m + 1) * P],
                            oT[:, k:k + 2, :], start=(k == 0),
                            stop=(k == KD - 2), perf_mode=DR)
                    nc.vector.scalar_tensor_tensor(
                        out=xpT[:, m, :], in0=ps[:, :TQ],
                        scalar=1.0 / (WS * XS), in1=xt[:, m, :],
                        op0=ALU.mult, op1=ALU.add)

                # ---------------- rmsnorm2 + router ----------------
                sq = aact.tile([P, KD, TQ], F32)
                rs = aact.tile([1, TQ], F32)
                sd = aact.tile([1, TQ], F32)
                rsb = aact.tile([P, TQ], F32)
                rstok = aact.tile([P, 2], F32)
                lg = aact.tile([P, 2, E], F32)
                comb = aact.tile([P, 2, E], F32)
                cT = aact.tile([E, TQ], F32)

                for m in range(KD):
                    nc.scalar.square(sq[:, m, :], xpT[:, m, :])
                psq = psD.tile([1, TQ], F32, tag="aux")
                for k in range(KD):
                    nc.tensor.matmul(psq, ones_cf[:, 0:1], sq[:, k, :],
                                     start=(k == 0), stop=(k == KD - 1))
                nc.scalar.activation(sd[0:1, :], psq, ACTF.Sqrt,
                                     bias=eps1[0:1, :], scale=1.0 / D)
                nc.vector.reciprocal(rs[0:1, :], sd[0:1, :])
                psb = psD.tile([P, TQ], F32, tag="aux")
                nc.tensor.matmul(psb, ones_row, rs[0:1, :], start=True, stop=True)
                nc.scalar.copy(rsb, psb)
                for m in range(KD):
                    nc.vector.scalar_tensor_tensor(
                        out=nx2[:, m, :], in0=xpT[:, m, :], scalar=XS,
                        in1=rsb, op0=ALU.mult, op1=ALU.mult)

                # rs in token-major via PE transpose (rows of rsb^T are
                # const). Dedicated psT pool: sharing psA here would make
                # the MoE psum tiles inherit a WAR hazard on the router
                # chain and stall the MoE start by ~7us.
                for t in range(2):
                    pst = psT.tile([P, P], F32, tag="tr")
                    nc.tensor.transpose(pst[:, :P], rsb[:, t * P:(t + 1) * P],
                                        ident)
                    nc.scalar.copy(rstok[:, t:t + 1], pst[:, 0:1])

                # logits^T [E, TQ] with Wr stationary (8-col LDWEIGHTS is
                # nearly free), then PE-transpose to token-major
                psrT = psD.tile([E, TQ], F32, tag="auxr")
                for k in range(KD):
                    nc.tensor.matmul(
                        psrT, wrr[:, k, :], xpT[:, k, :],
                        start=(k == 0), stop=(k == KD - 1))
                lgTs = aact.tile([E, TQ], F32)
                nc.scalar.copy(lgTs, psrT)
                for t in range(2):
                    pst2 = psT.tile([P, P], F32, tag="tr")
                    nc.tensor.transpose(pst2[:, :E],
                                        lgTs[:, t * P:(t + 1) * P],
                                        ident[:E, :E])
                    nc.vector.scalar_tensor_tensor(
                        out=lg[:, t, :], in0=pst2[:, :E],
                        scalar=rstok[:, t:t + 1],
                        in1=ebias_b, op0=ALU.mult, op1=ALU.add)

                # softmax over experts + top-2 renormalized combine weights
                for t in range(2):
                    lgt = lg[:, t, :]
                    m_s = aact.tile([P, 1], F32, tag="sm", name=f"m_{t}")
                    nc.vector.reduce_max(m_s, lgt, axis=AX)
                    negm = aact.tile([P, 1], F32, tag="sm2", name=f"nm_{t}")
                    nc.vector.tensor_scalar_mul(negm, m_s, -1.0)
                    e_s = aact.tile([P, E], F32, tag="sm3", name=f"e_{t}")
                    den = aact.tile([P, 1], F32, tag="sm4", name=f"d_{t}")
                    nc.scalar.activation(e_s, lgt, ACTF.Exp, bias=negm,
                                         accum_out=den)
                    rcp = aact.tile([P, 1], F32, tag="sm5", name=f"r_{t}")
                    nc.vector.reciprocal(rcp, den)
                    rw = aact.tile([P, E], F32, tag="sm6", name=f"rw_{t}")
                    nc.vector.tensor_scalar_mul(rw, e_s, rcp)
                    m1 = aact.tile([P, 1], F32, tag="sm7", name=f"m1_{t}")
                    nc.vector.reduce_max(m1, rw, axis=AX)
                    mask1 = aact.tile([P, E], F32, tag="sm8", name=f"k1_{t}")
                    nc.vector.tensor_scalar(mask1, rw, m1, None, ALU.is_ge)
                    rw2 = aact.tile([P, E], F32, tag="sm9", name=f"rw2_{t}")
                    nc.vector.scalar_tensor_tensor(
                        out=rw2, in0=mask1, scalar=-10.0, in1=rw,
                        op0=ALU.mult, op1=ALU.add)
                    m2 = aact.tile([P, 1], F32, tag="sm10", name=f"m2_{t}")
                    nc.vector.reduce_max(m2, rw2, axis=AX)
                    masktop = aact.tile([P, E], F32, tag="sm11", name=f"kt_{t}")
                    nc.vector.tensor_scalar(masktop, rw, m2, None, ALU.is_ge)
                    er = aact.tile([P, E], F32, tag="sm12", name=f"er_{t}")
                    nc.scalar.activation(er, rw, ACTF.Exp)
                    erm = aact.tile([P, E], F32, tag="sm13", name=f"em_{t}")
                    nc.vector.tensor_mul(out=erm, in0=er, in1=masktop)
                    den2 = aact.tile([P, 1], F32, tag="sm14", name=f"d2_{t}")
                    nc.vector.reduce_sum(den2, erm, axis=AX)
                    rcp2 = aact.tile([P, 1], F32, tag="sm15", name=f"r2_{t}")
                    nc.vector.reciprocal(rcp2, den2)
                    nc.vector.tensor_scalar_mul(comb[:, t, :], erm, rcp2)

                # combine^T [E, TQ] via PE transpose, then row-broadcast
                for t in range(2):
                    pst = psT.tile([P, P], F32, tag="tr")
                    nc.tensor.transpose(pst[:E, :P], comb[:, t, :], ident)
                    nc.scalar.copy(cT[:, t * P:(t + 1) * P], pst[:E, :P])
                with tc.tile_pool(name="dbounce", bufs=1, space="DRAM") as dbp:
                    cdram = dbp.tile([E, TQ], F32)
                    nc.sync.dma_start(cdram, cT)
                    for e in range(E):
                        nc.sync.dma_start(
                            cb[:, e, :],
                            cdram[e:e + 1, :].to_broadcast([P, TQ]))

            # ---------------- shared expert + 8 routed experts ----------------
            with (
                tc.tile_pool(name="wmoe", bufs=2) as wmoe,
                tc.tile_pool(name="mact", bufs=2) as mact,
                tc.tile_pool(name="psM", bufs=4, space="PSUM") as psM,
                tc.tile_pool(name="psO", bufs=3, space="PSUM") as psO,
            ):
                # fp8 scales: psa = WS*XS*h1, hp stores HS*h (and combine w
                # for routed experts), pso = WS*HS*out.
                S1 = 1.0 / (WS * XS)
                SH = HS / (WS * XS)
                SO = 1.0 / (WS * HS)
                for u in range(E + 1):
                    if u == 0:
                        w1, w3, w2 = w1_0, w3_0, w2_0
                    else:
                        w1 = wmoe.tile([P, KD, F], F8, tag="w1")
                        nc.sync.dma_start(w1, _r(din["ew1"][u - 1], F))
                        w3 = wmoe.tile([P, KD, F], F8, tag="w3")
                        nc.sync.dma_start(w3, _r(din["ew3"][u - 1], F))
                        w2 = wmoe.tile([P, KF, D], F8, tag="w2")
                        nc.sync.dma_start(w2, _r(din["ew2"][u - 1], D))

                    # m-chunks processed in pairs so ACT/DVE ops run at 512
                    # width: halves the per-unit DVE op count, which was the
                    # MoE bottleneck (PSUM-drain bound at 256 width)
                    hp = mact.tile([P, KF, TQ], F8, tag="hp")
                    for m in range(0, KF, 2):
                        psa = psM.tile([P, 2, TQ], F32, tag="ab")
                        for mm in range(2):
                            for k in range(0, KD, 2):
                                nc.tensor.matmul(
                                    psa[:, mm, :],
                                    w1[:, k:k + 2, (m + mm) * P:(m + mm + 1) * P],
                                    nx2[:, k:k + 2, :],
                                    start=(k == 0), stop=(k == KD - 2),
                                    perf_mode=DR)
                        sa = mact.tile([P, 2, TQ], BF16, tag="sa")
                        nc.scalar.activation(sa, psa, ACTF.Silu, scale=S1)
                        psb2 = psM.tile([P, 2, TQ], F32, tag="ab")
                        for mm in range(2):
                            for k in range(0, KD, 2):
                                nc.tensor.matmul(
                                    psb2[:, mm, :],
                                    w3[:, k:k + 2, (m + mm) * P:(m + mm + 1) * P],
                                    nx2[:, k:k + 2, :],
                                    start=(k == 0), stop=(k == KD - 2),
                                    perf_mode=DR)
                        if u == 0:
                            nc.vector.scalar_tensor_tensor(
                                out=hp[:, m:m + 2, :], in0=psb2, scalar=SH,
                                in1=sa, op0=ALU.mult, op1=ALU.mult)
                        else:
                            tmp = mact.tile([P, 2, TQ], BF16, tag="tmp")
                            for mm in range(2):
                                nc.vector.tensor_mul(
                                    out=tmp[:, mm, :], in0=sa[:, mm, :],
                                    in1=cb[:, u - 1, :])
                            nc.vector.scalar_tensor_tensor(
                                out=hp[:, m:m + 2, :], in0=psb2, scalar=SH,
                                in1=tmp, op0=ALU.mult, op1=ALU.mult)
                    for m in range(0, KD, 2):
                        pso = psO.tile([P, 2, TQ], F32, tag="eo")
                        for mm in range(2):
                            for k in range(0, KF, 2):
                                nc.tensor.matmul(
                                    pso[:, mm, :],
                                    w2[:, k:k + 2, (m + mm) * P:(m + mm + 1) * P],
                                    hp[:, k:k + 2, :],
                                    start=(k == 0), stop=(k == KF - 2),
                                    perf_mode=DR)
                        if u == 0:
                            nc.vector.tensor_scalar_mul(spec[:, m:m + 2, :],
                                                        pso, SO)
                        elif u < E:
                            nc.vector.scalar_tensor_tensor(
                                out=spec[:, m:m + 2, :], in0=pso, scalar=SO,
                                in1=spec[:, m:m + 2, :],
                                op0=ALU.mult, op1=ALU.add)
                        else:
                            nc.vector.scalar_tensor_tensor(
                                out=spec[:, m:m + 2, :], in0=pso, scalar=SO,
                                in1=spec[:, m:m + 2, :],
                                op0=ALU.mult, op1=ALU.add)
                            nc.vector.tensor_add(out=outT[:, m:m + 2, :],
                                                 in0=spec[:, m:m + 2, :],
                                                 in1=xpT[:, m:m + 2, :])
                            nc.sync.dma_start(
                                _r(outt[:], TQ)[:, m:m + 2, :],
                                outT[:, m:m + 2, :])

    nc.finalize()
    return nc


def _prep_inputs(inputs):
    bf = ml_dtypes.bfloat16
    f8 = ml_dtypes.float8_e4m3
    X = np.asarray(inputs["X"], np.float32)
    g1 = np.asarray(inputs["g1"], np.float32)
    rs1 = 1.0 / np.sqrt(np.mean(X * X, axis=-1, keepdims=True) + EPS)
    nx = X * rs1 * g1                      # [2, S, D] fp32
    nxT = np.clip(np.ascontiguousarray(np.transpose(nx, (0, 2, 1))) * XS,
                  -240.0, 240.0).astype(f8)
    XT = np.ascontiguousarray(np.transpose(X, (0, 2, 1)))

    def pm(a):
        """[C*P, N] -> partition-major [P, C*N]."""
        cp, n = a.shape
        return np.ascontiguousarray(
            a.reshape(cp // P, P, n).swapaxes(0, 1).reshape(P, -1))

    def t2(a):
        return pm(np.ascontiguousarray(np.asarray(a, np.float32).T))

    def t3(a):
        a = np.transpose(np.asarray(a, np.float32), (0, 2, 1))
        return np.stack([pm(np.ascontiguousarray(a[e])) for e in range(E)])

    f8 = ml_dtypes.float8_e4m3

    def q8(a):
        """fp8e4 quantize with the WS weight scale (clip to TRN max 240)."""
        return np.clip(a * WS, -240.0, 240.0).astype(f8)

    shared = {
        "wq": q8(t2(inputs["Wq"])),
        "wkc": q8(t2(inputs["Wkc"])),
        "wvc": q8(t2(inputs["Wvc"])),
        "wo": q8(t2(inputs["Wo"])),
        "wr": t2(inputs["Wr"]).astype(np.float32),
        "ebias": np.asarray(inputs["expert_bias"],
                            np.float32).reshape(1, E),
        "sw1": q8(t2(inputs["sW1"])),
        "sw3": q8(t2(inputs["sW3"])),
        "sw2": q8(t2(inputs["sW2"])),
        "ew1": q8(t3(inputs["eW1"])),
        "ew3": q8(t3(inputs["eW3"])),
        "ew2": q8(t3(inputs["eW2"])),
    }
    in_maps = []
    for c in range(8):
        b, q0 = c // 4, (c % 4) * TQ
        m = dict(shared)

        def pm(a):
            cp, n = a.shape
            return np.ascontiguousarray(
                a.reshape(cp // P, P, n).swapaxes(0, 1).reshape(P, -1))

        m["nxt"] = pm(nxT[b])
        m["nxq"] = pm(np.ascontiguousarray(nxT[b][:, q0:q0 + TQ]))
        m["xt"] = pm(np.ascontiguousarray(XT[b][:, q0:q0 + TQ]))
        in_maps.append(m)
    return in_maps


def run_on_device(inputs, trace=False):
    if "nc" not in _CACHE:
        _CACHE["nc"] = build_program()
    nc = _CACHE["nc"]
    in_maps = _prep_inputs(inputs)
    res = run_bass_kernel_spmd(nc, in_maps, core_ids=list(range(8)),
                               trace=trace)
    out = np.empty((2, S, D), np.float32)
    for c in range(8):
        b, q0 = c // 4, (c % 4) * TQ
        ot = res.results[c]["outt"].reshape(P, KD, TQ).swapaxes(0, 1)
        out[b, q0:q0 + TQ, :] = ot.reshape(D, TQ).T
    return out, res


def kernel(**inputs):
    out, _ = run_on_device(inputs, trace=False)
    return out



# revision 28
# speedup vs baseline: 1.5703x; 1.0063x over previous
"""DeepSeek layer (MLA attention + shared/routed MoE) on 8 TRN2 NeuronCores.

Data-parallel over tokens: core c handles batch c//4, tokens [(c%4)*256, ...).
Activations live feature-major [feature, token] on device; host pre-transposes
weights (bf16) and precomputes the first rmsnorm (depends only on input X).
Router logits are computed in fp32 so top-2 expert selection matches the
reference; expert matmuls run in bf16.
"""

import numpy as np
import ml_dtypes

import concourse.bass as bass
import concourse.tile as tile
from concourse import bacc, mybir
from concourse.bass_utils import run_bass_kernel_spmd
from concourse.masks import make_identity

BF16 = mybir.dt.bfloat16
F32 = mybir.dt.float32
F8 = mybir.dt.float8e4
AX = mybir.AxisListType.X
ALU = mybir.AluOpType
ACTF = mybir.ActivationFunctionType
DR = mybir.MatmulPerfMode.DoubleRow

# fp8 scale factors: weights x64, activations x16, hidden x8
WS = 64.0
XS = 16.0
HS = 8.0

P = 128
D = 1024
KD = D // P          # 8 feature chunks
S = 1024             # keys per batch
TQ = 256             # query tokens per core
H = 4
DC = 256             # compressed kv dim == dk
F = 1024
KF = F // P
E = 8
EPS = 1e-6
SCALE = 1.0 / 16.0   # 1/sqrt(dk)

_CACHE = {}


def _r(ap, n=None):
    """Host-permuted DRAM [P, C*N] -> [P, C, N] view (contiguous)."""
    c = ap.shape[-1]
    n = n if n is not None else c // KD
    return ap.rearrange("p (k n) -> p k n", n=n)


def build_program():
    nc = bacc.Bacc(None)

    # All tensors are host-permuted to partition-major [P, chunks*N] so each
    # DMA is one contiguous segment per partition (descriptor-rate matters).
    din = {}
    F32R = mybir.dt.float32r
    for name, shape, dt in [
        ("nxt", [P, KD * S], F8),
        ("nxq", [P, KD * TQ], F8),
        ("xt", [P, KD * TQ], F32),
        ("wq", [P, KD * D], F8),
        ("wkc", [P, KD * DC], F8),
        ("wvc", [P, KD * DC], F8),
        ("wo", [P, KD * D], F8),
        ("wr", [P, KD * E], F32),
        ("ebias", [1, E], F32),
        ("sw1", [P, KD * F], F8),
        ("sw3", [P, KD * F], F8),
        ("sw2", [P, KF * D], F8),
        ("ew1", [E, P, KD * F], F8),
        ("ew3", [E, P, KD * F], F8),
        ("ew2", [E, P, KF * D], F8),
    ]:
        din[name] = nc.dram_tensor(name, shape, dt, kind="ExternalInput")
    outt = nc.dram_tensor("outt", [P, KD * TQ], F32, kind="ExternalOutput")

    with tile.TileContext(nc) as tc:
        with (
            tc.tile_pool(name="const", bufs=1) as const,
            tc.tile_pool(name="persist", bufs=1) as persist,
        ):
            ones_f8 = const.tile([P, 1], F8)
            nc.vector.memset(ones_f8, 1.0)
            ones_cf = const.tile([P, 1], F32)
            nc.vector.memset(ones_cf, 1.0)
            ones_row = const.tile([1, P], F32)
            nc.vector.memset(ones_row, 1.0)
            eps1 = const.tile([1, 1], F32)
            nc.vector.memset(eps1, EPS)
            ident = const.tile([P, P], F32)
            make_identity(nc, ident)
            ebias_b = const.tile([P, E], F32)
            nc.sync.dma_start(ebias_b, din["ebias"][:].to_broadcast([P, E]))
            # warm all three activation tables (Exp/Sqrt/Silu) up front so
            # the 1.3us table loads overlap the input DMA instead of the
            # serial rmsnorm/router path
            warm = const.tile([1, 4], F32)
            nc.scalar.activation(warm[0:1, 0:1], ones_cf[0:1, 0:1], ACTF.Exp)
            nc.scalar.activation(warm[0:1, 1:2], ones_cf[0:1, 0:1], ACTF.Sqrt)
            nc.scalar.activation(warm[0:1, 2:3], ones_cf[0:1, 0:1],
                                 ACTF.Silu)

            xpT = persist.tile([P, KD, TQ], F32)      # X' = X + attn out
            nx2 = persist.tile([P, KD, TQ], F8)       # rmsnorm2(X')*XS fp8
            cb = persist.tile([P, E, TQ], F32)        # combine weights bcast
            spec = persist.tile([P, KD, TQ], F32)     # shared+experts accum
            outT = persist.tile([P, KD, TQ], F32)

            # shared-expert weights, prefetched during attention so the MoE
            # phase starts without a DMA stall (DMAs issued after the
            # attention inputs below)
            w1_0 = persist.tile([P, KD, F], F8)
            w3_0 = persist.tile([P, KD, F], F8)
            w2_0 = persist.tile([P, KF, D], F8)

            # ---------------- attention ----------------
            with (
                tc.tile_pool(name="ainp", bufs=1) as ainp,
                tc.tile_pool(name="awts", bufs=1) as awts,
                tc.tile_pool(name="aact", bufs=1) as aact,
                tc.tile_pool(name="psA", bufs=3, space="PSUM") as psA,
                tc.tile_pool(name="psD", bufs=2, space="PSUM") as psD,
                tc.tile_pool(name="psT", bufs=1, space="PSUM") as psT,
            ):
                # DMA issue order: Q-path operands first so PE starts early
                nxq = ainp.tile([P, KD, TQ], F8)
                nc.sync.dma_start(nxq, _r(din["nxq"][:], TQ))
                wq = awts.tile([P, KD, D], F8)
                nc.sync.dma_start(wq[:, :, :512], _r(din["wq"][:], D)[:, :, :512])
                nc.sync.dma_start(wq[:, :, 512:], _r(din["wq"][:], D)[:, :, 512:])
                wkc = awts.tile([P, KD, DC], F8)
                nc.sync.dma_start(wkc, _r(din["wkc"][:], DC))
                nxt = ainp.tile([P, KD, S], F8)
                nc.sync.dma_start(nxt[:, :, :512], _r(din["nxt"][:], S)[:, :, :512])
                nc.sync.dma_start(nxt[:, :, 512:], _r(din["nxt"][:], S)[:, :, 512:])
                wvc = awts.tile([P, KD, DC], F8)
                nc.sync.dma_start(wvc, _r(din["wvc"][:], DC))
                xt = ainp.tile([P, KD, TQ], F32)
                nc.sync.dma_start(xt, _r(din["xt"][:], TQ))
                wo = awts.tile([P, KD, D], F8)
                nc.sync.dma_start(wo, _r(din["wo"][:], D))
                wrr = awts.tile([P, KD, E], F32)
                nc.sync.dma_start(wrr, _r(din["wr"][:], E))

                qT = aact.tile([P, KD, TQ], F8)
                kcT = aact.tile([P, 2, S], F8)
                vc = aact.tile([P, KD, DC], F8)
                probs = aact.tile([P, KD, H, TQ], F8)
                rcpd = aact.tile([1, H, TQ], F32)
                rcpb = aact.tile([P, H, TQ], F32)
                oT = aact.tile([P, KD, TQ], F8)

                # Q^T [d, tq] = Wq @ nxq; psum = WS*XS*q, copy back at XS*q
                for m in range(KD):
                    ps = psA.tile([P, 512], F32, tag="mm")
                    for k in range(0, KD, 2):
                        nc.tensor.matmul(
                            ps[:, :TQ], wq[:, k:k + 2, m * P:(m + 1) * P],
                            nxq[:, k:k + 2, :], start=(k == 0),
                            stop=(k == KD - 2), perf_mode=DR)
                    nc.scalar.activation(qT[:, m, :], ps[:, :TQ], ACTF.Copy,
                                         scale=1.0 / WS)

                # Kc^T [dc, S] = Wkc @ nxt (kept at XS*kc)
                for m in range(2):
                    for n2 in range(2):
                        ps = psA.tile([P, 512], F32, tag="mm")
                        for k in range(0, KD, 2):
                            nc.tensor.matmul(
                                ps, wkc[:, k:k + 2, m * P:(m + 1) * P],
                                nxt[:, k:k + 2, n2 * 512:(n2 + 1) * 512],
                                start=(k == 0), stop=(k == KD - 2),
                                perf_mode=DR)
                        nc.scalar.activation(
                            kcT[:, m, n2 * 512:(n2 + 1) * 512], ps, ACTF.Copy,
                            scale=1.0 / WS)

                # Vc [S, dc] = nxt^T @ Wvc^T  (keys-major, XS*vc)
                for kc in range(KD):
                    ps = psA.tile([P, 512], F32, tag="mm")
                    for k in range(0, KD, 2):
                        nc.tensor.matmul(
                            ps[:, :DC], nxt[:, k:k + 2, kc * P:(kc + 1) * P],
                            wvc[:, k:k + 2, :], start=(k == 0),
                            stop=(k == KD - 2), perf_mode=DR)
                    nc.scalar.activation(vc[:, kc, :], ps[:, :DC], ACTF.Copy,
                                         scale=1.0 / WS)

                # scores^T + exp (keys-major, no max-sub needed); psum holds
                # XS*XS*s so fold 1/XS^2 into the exp scale
                for h in range(H):
                    for kc in range(KD):
                        ps = psA.tile([P, 512], F32, tag="mm")
                        nc.tensor.matmul(
                            ps[:, :TQ], kcT[:, 0:2, kc * P:(kc + 1) * P],
                            qT[:, 2 * h:2 * h + 2, :], start=True, stop=True,
                            perf_mode=DR)
                        nc.scalar.activation(
                            probs[:, kc, h, :], ps[:, :TQ], ACTF.Exp,
                            scale=SCALE / (XS * XS))

                # prefetch shared-expert weights now: attention inputs are
                # in flight, MoE phase is ~50us away
                nc.sync.dma_start(w1_0, _r(din["sw1"][:], F))
                nc.sync.dma_start(w3_0, _r(din["sw3"][:], F))
                nc.sync.dma_start(w2_0, _r(din["sw2"][:], D))

                # softmax denominators + reciprocal + broadcast
                for h in range(H):
                    psd = psD.tile([1, TQ], F32, tag="aux")
                    for kc in range(KD):
                        nc.tensor.matmul(
                            psd, ones_f8[:, 0:1], probs[:, kc, h, :],
                            start=(kc == 0), stop=(kc == KD - 1))
                    nc.vector.reciprocal(rcpd[0:1, h, :], psd)
                    psb = psD.tile([P, TQ], F32, tag="aux")
                    nc.tensor.matmul(psb, ones_row, rcpd[0:1, h, :],
                                     start=True, stop=True)
                    nc.scalar.copy(rcpb[:, h, :], psb)

                # out_h^T = Vc^T @ probs^T, normalized per token (XS*out)
                for h in range(H):
                    for m in range(2):
                        ps = psA.tile([P, 512], F32, tag="mm")
                        for kc in range(0, KD, 2):
                            nc.tensor.matmul(
                                ps[:, :TQ],
                                vc[:, kc:kc + 2, m * P:(m + 1) * P],
                                probs[:, kc:kc + 2, h, :],
                                start=(kc == 0), stop=(kc == KD - 2),
                                perf_mode=DR)
                        nc.vector.tensor_mul(
                            out=oT[:, 2 * h + m, :], in0=ps[:, :TQ],
                            in1=rcpb[:, h, :])

                # attn proj + residual: X' = Wo @ O / (WS*XS) + X
                for m in range(KD):
                    ps = psA.tile([P, 512], F32, tag="mm")
                    for k in range(0, KD, 2):
                        nc.tensor.matmul(
                            ps[:, :TQ], wo[:, k:k + 2, m * P:(m + 1) * P],
                            oT[:, k:k + 2, :], start=(k == 0),
                            stop=(k == KD - 2), perf_mode=DR)
                    nc.vector.scalar_tensor_tensor(
                        out=xpT[:, m, :], in0=ps[:, :TQ],
                        scalar=1.0 / (WS * XS), in1=xt[:, m, :],
                        op0=ALU.mult, op1=ALU.add)

                # ---------------- rmsnorm2 + router ----------------
                sq = aact.tile([P, KD, TQ], F32)
                rs = aact.tile([1, TQ], F32)
                sd = aact.tile([1, TQ], F32)
                rsb = aact.tile([P, TQ], F32)
                rstok = aact.tile([P, 2], F32)
                lg = aact.tile([P, 2, E], F32)
                comb = aact.tile([P, 2, E], F32)
                cT = aact.tile([E, TQ], F32)

                for m in range(KD):
                    nc.scalar.square(sq[:, m, :], xpT[:, m, :])
                psq = psD.tile([1, TQ], F32, tag="aux")
                for k in range(KD):
                    nc.tensor.matmul(psq, ones_cf[:, 0:1], sq[:, k, :],
                                     start=(k == 0), stop=(k == KD - 1))
                nc.scalar.activation(sd[0:1, :], psq, ACTF.Sqrt,
                                     bias=eps1[0:1, :], scale=1.0 / D)
                nc.vector.reciprocal(rs[0:1, :], sd[0:1, :])
                psb = psD.tile([P, TQ], F32, tag="aux")
                nc.tensor.matmul(psb, ones_row, rs[0:1, :], start=True, stop=True)
                nc.scalar.copy(rsb, psb)
                for m in range(KD):
                    nc.vector.scalar_tensor_tensor(
                        out=nx2[:, m, :], in0=xpT[:, m, :], scalar=XS,
                        in1=rsb, op0=ALU.mult, op1=ALU.mult)

                # rs in token-major via PE transpose (rows of rsb^T are
                # const). Dedicated psT pool: sharing psA here would make
                # the MoE psum tiles inherit a WAR hazard on the router
                # chain and stall the MoE start by ~7us.
                for t in range(2):
                    pst = psT.tile([P, P], F32, tag="tr")
                    nc.tensor.transpose(pst[:, :P], rsb[:, t * P:(t + 1) * P],
                                        ident)
                    nc.scalar.copy(rstok[:, t:t + 1], pst[:, 0:1])

                # logits^T [E, TQ] with Wr stationary (8-col LDWEIGHTS is
                # nearly free), then PE-transpose to token-major
                psrT = psD.tile([E, TQ], F32, tag="auxr")
                for k in range(KD):
                    nc.tensor.matmul(
                        psrT, wrr[:, k, :], xpT[:, k, :],
                        start=(k == 0), stop=(k == KD - 1))
                lgTs = aact.tile([E, TQ], F32)
                nc.scalar.copy(lgTs, psrT)
                for t in range(2):
                    pst2 = psT.tile([P, P], F32, tag="tr")
                    nc.tensor.transpose(pst2[:, :E],
                                        lgTs[:, t * P:(t + 1) * P],
                                        ident[:E, :E])
                    nc.vector.scalar_tensor_tensor(
                        out=lg[:, t, :], in0=pst2[:, :E],
                        scalar=rstok[:, t:t + 1],
                        in1=ebias_b, op0=ALU.mult, op1=ALU.add)

                # softmax over experts + top-2 renormalized combine weights
                for t in range(2):
                    lgt = lg[:, t, :]
                    m_s = aact.tile([P, 1], F32, tag="sm", name=f"m_{t}")
                    nc.vector.reduce_max(m_s, lgt, axis=AX)
                    negm = aact.tile([P, 1], F32, tag="sm2", name=f"nm_{t}")
                    nc.vector.tensor_scalar_mul(negm, m_s, -1.0)
                    e_s = aact.tile([P, E], F32, tag="sm3", name=f"e_{t}")
                    den = aact.tile([P, 1], F32, tag="sm4", name=f"d_{t}")
                    nc.scalar.activation(e_s, lgt, ACTF.Exp, bias=negm,
                                         accum_out=den)
                    rcp = aact.tile([P, 1], F32, tag="sm5", name=f"r_{t}")
                    nc.vector.reciprocal(rcp, den)
                    rw = aact.tile([P, E], F32, tag="sm6", name=f"rw_{t}")
                    nc.vector.tensor_scalar_mul(rw, e_s, rcp)
                    m1 = aact.tile([P, 1], F32, tag="sm7", name=f"m1_{t}")
                    nc.vector.reduce_max(m1, rw, axis=AX)
                    mask1 = aact.tile([P, E], F32, tag="sm8", name=f"k1_{t}")
                    nc.vector.tensor_scalar(mask1, rw, m1, None, ALU.is_ge)
                    rw2 = aact.tile([P, E], F32, tag="sm9", name=f"rw2_{t}")
                    nc.vector.scalar_tensor_tensor(
                        out=rw2, in0=mask1, scalar=-10.0, in1=rw,
                        op0=ALU.mult, op1=ALU.add)
                    m2 = aact.tile([P, 1], F32, tag="sm10", name=f"m2_{t}")
                    nc.vector.reduce_max(m2, rw2, axis=AX)
                    masktop = aact.tile([P, E], F32, tag="sm11", name=f"kt_{t}")
                    nc.vector.tensor_scalar(masktop, rw, m2, None, ALU.is_ge)
                    er = aact.tile([P, E], F32, tag="sm12", name=f"er_{t}")
                    nc.scalar.activation(er, rw, ACTF.Exp)
                    erm = aact.tile([P, E], F32, tag="sm13", name=f"em_{t}")
                    nc.vector.tensor_mul(out=erm, in0=er, in1=masktop)
                    den2 = aact.tile([P, 1], F32, tag="sm14", name=f"d2_{t}")
                    nc.vector.reduce_sum(den2, erm, axis=AX)
                    rcp2 = aact.tile([P, 1], F32, tag="sm15", name=f"r2_{t}")
                    nc.vector.reciprocal(rcp2, den2)
                    nc.vector.tensor_scalar_mul(comb[:, t, :], erm, rcp2)

                # combine^T [E, TQ] via PE transpose, then row-broadcast
                for t in range(2):
                    pst = psT.tile([P, P], F32, tag="tr")
                    nc.tensor.transpose(pst[:E, :P], comb[:, t, :], ident)
                    nc.scalar.copy(cT[:, t * P:(t + 1) * P], pst[:E, :P])
                with tc.tile_pool(name="dbounce", bufs=1, space="DRAM") as dbp:
                    cdram = dbp.tile([E, TQ], F32)
                    nc.sync.dma_start(cdram, cT)
                    for e in range(E):
                        nc.sync.dma_start(
                            cb[:, e, :],
                            cdram[e:e + 1, :].to_broadcast([P, TQ]))

            # ---------------- shared expert + 8 routed experts ----------------
            with (
                tc.tile_pool(name="wmoe", bufs=2) as wmoe,
                tc.tile_pool(name="mact", bufs=2) as mact,
                tc.tile_pool(name="psM", bufs=4, space="PSUM") as psM,
                tc.tile_pool(name="psO", bufs=3, space="PSUM") as psO,
            ):
                # fp8 scales: psa = WS*XS*h1, hp stores HS*h (and combine w
                # for routed experts), pso = WS*HS*out.
                S1 = 1.0 / (WS * XS)
                SH = HS / (WS * XS)
                SO = 1.0 / (WS * HS)
                for u in range(E + 1):
                    if u == 0:
                        w1, w3, w2 = w1_0, w3_0, w2_0
                    else:
                        w1 = wmoe.tile([P, KD, F], F8, tag="w1")
                        nc.sync.dma_start(w1, _r(din["ew1"][u - 1], F))
                        w3 = wmoe.tile([P, KD, F], F8, tag="w3")
                        nc.sync.dma_start(w3, _r(din["ew3"][u - 1], F))
                        w2 = wmoe.tile([P, KF, D], F8, tag="w2")
                        nc.sync.dma_start(w2, _r(din["ew2"][u - 1], D))

                    # m-chunks processed in pairs so ACT/DVE ops run at 512
                    # width: halves the per-unit DVE op count, which was the
                    # MoE bottleneck (PSUM-drain bound at 256 width)
                    hp = mact.tile([P, KF, TQ], F8, tag="hp")
                    for m in range(0, KF, 2):
                        psa = psM.tile([P, 2, TQ], F32, tag="ab")
                        for mm in range(2):
                            for k in range(0, KD, 2):
                                nc.tensor.matmul(
                                    psa[:, mm, :],
                                    w1[:, k:k + 2, (m + mm) * P:(m + mm + 1) * P],
                                    nx2[:, k:k + 2, :],
                                    start=(k == 0), stop=(k == KD - 2),
                                    perf_mode=DR)
                        sa = mact.tile([P, 2, TQ], BF16, tag="sa")
                        nc.scalar.activation(sa, psa, ACTF.Silu, scale=S1)
                        psb2 = psM.tile([P, 2, TQ], F32, tag="ab")
                        for mm in range(2):
                            for k in range(0, KD, 2):
                                nc.tensor.matmul(
                                    psb2[:, mm, :],
                                    w3[:, k:k + 2, (m + mm) * P:(m + mm + 1) * P],
                                    nx2[:, k:k + 2, :],
                                    start=(k == 0), stop=(k == KD - 2),
                                    perf_mode=DR)
                        if u == 0:
                            nc.vector.scalar_tensor_tensor(
                                out=hp[:, m:m + 2, :], in0=psb2, scalar=SH,
                                in1=sa, op0=ALU.mult, op1=ALU.mult)
                        else:
                            # gating multiply on the otherwise-idle gpsimd
                            # engine: DVE is the MoE-phase bottleneck
                            tmp = mact.tile([P, 2, TQ], BF16, tag="tmp")
                            for mm in range(2):
                                nc.gpsimd.tensor_mul(
                                    out=tmp[:, mm, :], in0=sa[:, mm, :],
                                    in1=cb[:, u - 1, :])
                            nc.vector.scalar_tensor_tensor(
                                out=hp[:, m:m + 2, :], in0=psb2, scalar=SH,
                                in1=tmp, op0=ALU.mult, op1=ALU.mult)
                    for m in range(0, KD, 2):
                        pso = psO.tile([P, 2, TQ], F32, tag="eo")
                        for mm in range(2):
                            for k in range(0, KF, 2):
                                nc.tensor.matmul(
                                    pso[:, mm, :],
                                    w2[:, k:k + 2, (m + mm) * P:(m + mm + 1) * P],
                                    hp[:, k:k + 2, :],
                                    start=(k == 0), stop=(k == KF - 2),
                                    perf_mode=DR)
                        if u == 0:
                            nc.vector.tensor_scalar_mul(spec[:, m:m + 2, :],
                                                        pso, SO)
                        elif u < E:
                            nc.vector.scalar_tensor_tensor(
                                out=spec[:, m:m + 2, :], in0=pso, scalar=SO,
                                in1=spec[:, m:m + 2, :],
                                op0=ALU.mult, op1=ALU.add)
                        else:
                            nc.vector.scalar_tensor_tensor(
                                out=spec[:, m:m + 2, :], in0=pso, scalar=SO,
                                in1=spec[:, m:m + 2, :],
                                op0=ALU.mult, op1=ALU.add)
                            nc.vector.tensor_add(out=outT[:, m:m + 2, :],
                                                 in0=spec[:, m:m + 2, :],
                                                 in1=xpT[:, m:m + 2, :])
                            nc.sync.dma_start(
                                _r(outt[:], TQ)[:, m:m + 2, :],
                                outT[:, m:m + 2, :])

    nc.finalize()
    return nc


def _prep_inputs(inputs):
    bf = ml_dtypes.bfloat16
    f8 = ml_dtypes.float8_e4m3
    X = np.asarray(inputs["X"], np.float32)
    g1 = np.asarray(inputs["g1"], np.float32)
    rs1 = 1.0 / np.sqrt(np.mean(X * X, axis=-1, keepdims=True) + EPS)
    nx = X * rs1 * g1                      # [2, S, D] fp32
    nxT = np.clip(np.ascontiguousarray(np.transpose(nx, (0, 2, 1))) * XS,
                  -240.0, 240.0).astype(f8)
    XT = np.ascontiguousarray(np.transpose(X, (0, 2, 1)))

    def pm(a):
        """[C*P, N] -> partition-major [P, C*N]."""
        cp, n = a.shape
        return np.ascontiguousarray(
            a.reshape(cp // P, P, n).swapaxes(0, 1).reshape(P, -1))

    def t2(a):
        return pm(np.ascontiguousarray(np.asarray(a, np.float32).T))

    def t3(a):
        a = np.transpose(np.asarray(a, np.float32), (0, 2, 1))
        return np.stack([pm(np.ascontiguousarray(a[e])) for e in range(E)])

    f8 = ml_dtypes.float8_e4m3

    def q8(a):
        """fp8e4 quantize with the WS weight scale (clip to TRN max 240)."""
        return np.clip(a * WS, -240.0, 240.0).astype(f8)

    shared = {
        "wq": q8(t2(inputs["Wq"])),
        "wkc": q8(t2(inputs["Wkc"])),
        "wvc": q8(t2(inputs["Wvc"])),
        "wo": q8(t2(inputs["Wo"])),
        "wr": t2(inputs["Wr"]).astype(np.float32),
        "ebias": np.asarray(inputs["expert_bias"],
                            np.float32).reshape(1, E),
        "sw1": q8(t2(inputs["sW1"])),
        "sw3": q8(t2(inputs["sW3"])),
        "sw2": q8(t2(inputs["sW2"])),
        "ew1": q8(t3(inputs["eW1"])),
        "ew3": q8(t3(inputs["eW3"])),
        "ew2": q8(t3(inputs["eW2"])),
    }
    in_maps = []
    for c in range(8):
        b, q0 = c // 4, (c % 4) * TQ
        m = dict(shared)

        def pm(a):
            cp, n = a.shape
            return np.ascontiguousarray(
                a.reshape(cp // P, P, n).swapaxes(0, 1).reshape(P, -1))

        m["nxt"] = pm(nxT[b])
        m["nxq"] = pm(np.ascontiguousarray(nxT[b][:, q0:q0 + TQ]))
        m["xt"] = pm(np.ascontiguousarray(XT[b][:, q0:q0 + TQ]))
        in_maps.append(m)
    return in_maps


def run_on_device(inputs, trace=False):
    if "nc" not in _CACHE:
        _CACHE["nc"] = build_program()
    nc = _CACHE["nc"]
    in_maps = _prep_inputs(inputs)
    res = run_bass_kernel_spmd(nc, in_maps, core_ids=list(range(8)),
                               trace=trace)
    out = np.empty((2, S, D), np.float32)
    for c in range(8):
        b, q0 = c // 4, (c % 4) * TQ
        ot = res.results[c]["outt"].reshape(P, KD, TQ).swapaxes(0, 1)
        out[b, q0:q0 + TQ, :] = ot.reshape(D, TQ).T
    return out, res


def kernel(**inputs):
    out, _ = run_on_device(inputs, trace=False)
    return out

